# revision 1
# baseline (speedup 1.0000x reference)
"""Trainium2 Bass kernel for nn_DecodingLoss (cepstrum decoding loss).

Math (per 4096-sample window):
  cep = irfft(log(|rfft(x)| + eps))[DELAYS]; softargmax(beta=1e10) -> argmax idx;
  loss = clip(|idx - symbol|,0,1); per-audio sums -> 5 scalar outputs.

Kernel strategy (8 cores, pure data parallel over the batch dim):
  FFT 4096 = 32 x 128 Cooley-Tukey: n = 128*t + s  (t<32, s<128)
    stage1 (PE): A[u,s] = sum_t x[128t+s] W32^{tu}  - block-diag(4 windows) C32/S32
    corner turn (PE transpose)
    stage2 (PE): X[k=u+32v] = sum_s A[u,s] W4096^{s(u+32v)}  - twiddle folded into
      per-u stationary H_u[s,v]; only k=1..2048 computed (hermitian; k=0 dropped -
      a uniform shift of all cep taps cancels in softmax exactly).
  log|X|: L = 0.5*log(Xre^2+Xim^2+1e-10) (ACT), then cep taps via one PE projection
  (delays are multiples of 32 -> cos tables fold), softargmax + loss on DVE/ACT.
  Host: sums per-audio errors and mirrors the reference's final scalar math.
"""
import numpy as np
import ml_dtypes

import concourse.bass as bass
import concourse.mybir as mybir
from concourse import tile
from concourse.bass_utils import run_bass_kernel_spmd

FP32 = mybir.dt.float32
F32R = mybir.dt.float32r
BF16 = mybir.dt.bfloat16
I32 = mybir.dt.int32

B, NW, WIN = 64, 128, 4096
NCORES = 8
BLOC = B // NCORES              # 8 audio rows per core
WLOC = BLOC * NW                # 1024 windows per core
T, S, U = 32, 128, 32           # n = 128 t + s ; k = u + 32 v
NV = 64                         # v-grid size per u
ITERS = 4
WPI = WLOC // ITERS             # 256 windows per iteration
G = WPI // 4                    # 64 groups of 4 windows
DELAYS = np.array([64, 96, 128, 160, 192, 224, 256, 288])
BETA = 1e10

_cache = {}


def _hoist_waits(bir_json):
    """This walrus build rejects instructions carrying attached semaphore waits
    ("Too many sync wait commands"); raw-bass style standalone EventSemaphore
    waits compile and run. Hoist every attached wait into its own
    EventSemaphore on the same engine queue; updates stay attached."""
    import json
    d = json.loads(bir_json)
    n = 0
    for fn in d["functions"]:
        for bb in fn["blocks"]:
            out = []
            for ins in bb["instructions"]:
                si = ins.get("sync_info")
                waits = (si or {}).get("on_wait") or []
                if waits and ins.get("opcode") != "EventSemaphore" and ins.get("engine"):
                    for w in waits:
                        n += 1
                        out.append({
                            "name": f"hoistw-{n}", "opcode": "EventSemaphore",
                            "engine": ins["engine"], "ins": [], "outs": [],
                            "sync_info": {"on_wait": [w], "on_update": []},
                        })
                    si["on_wait"] = []
                out.append(ins)
            bb["instructions"] = out
    return json.dumps(d).encode()


def _install_hoist(nc):
    orig = nc.to_json_bytes
    nc.to_json_bytes = lambda: _hoist_waits(orig())
    return nc
LINEARIZE = False


def _tables():
    t = np.arange(T)[:, None]
    u = np.arange(U)[None, :]
    c32 = np.cos(2 * np.pi * t * u / 32.0)
    s32n = -np.sin(2 * np.pi * t * u / 32.0)
    bdc = np.zeros((128, 128), np.float64)
    bds = np.zeros((128, 128), np.float64)
    for w in range(4):
        bdc[w * 32:w * 32 + 32, w * 32:w * 32 + 32] = c32
        bds[w * 32:w * 32 + 32, w * 32:w * 32 + 32] = s32n

    # k-grid per u: u==0 -> k = 32*(j+1) (j=0..63), else k = u + 32*j
    kgrid = np.zeros((U, NV), np.int64)
    kgrid[0] = 32 * (np.arange(NV) + 1)
    for uu in range(1, U):
        kgrid[uu] = uu + 32 * np.arange(NV)

    s = np.arange(S)[:, None]
    h2 = np.zeros((S, U, 3, NV), np.float64)
    for uu in range(U):
        ph = 2 * np.pi * s * kgrid[uu][None, :] / 4096.0
        h2[:, uu, 0] = np.cos(ph)           # Hre
        h2[:, uu, 1] = -np.sin(ph)          # Him
        h2[:, uu, 2] = np.sin(ph)           # -Him
    # projection: cep[d] = sum_k wk*0.5*log(m2)[k]*cos(2 pi k d/4096)/4096
    pp = np.zeros((128, 16, 8), np.float64)
    for p in range(16):
        for half in range(2):
            uu = 2 * p + half
            k = kgrid[uu]
            wk = np.where(k == 2048, 1.0, 2.0)
            for j, d in enumerate(DELAYS):
                pp[half * 64:half * 64 + 64, p, j] = (
                    wk * 0.5 * np.cos(2 * np.pi * k * d / 4096.0) / 4096.0)
    ident = np.eye(128)
    idxt = np.broadcast_to(np.arange(8.0), (128, 8)).copy()
    return (bdc.astype(ml_dtypes.bfloat16), bds.astype(ml_dtypes.bfloat16),
            h2.astype(ml_dtypes.bfloat16), pp.astype(np.float32),
            ident.astype(ml_dtypes.bfloat16), idxt.astype(np.float32))


def _build():
    nc = bass.Bass()
    audio = nc.dram_tensor("audio", [WLOC, WIN], BF16, kind="ExternalInput")
    syms = nc.dram_tensor("syms", [WLOC], I32, kind="ExternalInput")
    bdc_d = nc.dram_tensor("bdc", [128, 128], BF16, kind="ExternalInput")
    bds_d = nc.dram_tensor("bds", [128, 128], BF16, kind="ExternalInput")
    h2_d = nc.dram_tensor("h2", [S, U, 3, NV], BF16, kind="ExternalInput")
    pp_d = nc.dram_tensor("pp", [128, 16, 8], F32R, kind="ExternalInput")
    id_d = nc.dram_tensor("ident", [128, 128], BF16, kind="ExternalInput")
    ix_d = nc.dram_tensor("idxt", [128, 8], FP32, kind="ExternalInput")
    idf_d = nc.dram_tensor("identf", [128, 128], FP32, kind="ExternalInput")
    loss_out = nc.dram_tensor("loss_out", [WLOC], FP32, kind="ExternalOutput")

    with tile.TileContext(nc, linearize=LINEARIZE) as tc:
        with (
            tc.tile_pool(name="consts", bufs=1) as consts,
            tc.tile_pool(name="xt", bufs=2) as xt_pool,
            tc.tile_pool(name="as_", bufs=4) as as_pool,
            tc.tile_pool(name="at", bufs=2) as at_pool,
            tc.tile_pool(name="sq", bufs=2) as sq_pool,
            tc.tile_pool(name="m2", bufs=2) as m2_pool,
            tc.tile_pool(name="lg", bufs=2) as lg_pool,
            tc.tile_pool(name="fin", bufs=2) as fin_pool,
            tc.tile_pool(name="psA", bufs=2, space="PSUM") as psA_pool,
            tc.tile_pool(name="psT", bufs=2, space="PSUM") as psT_pool,
            tc.tile_pool(name="psX", bufs=2, space="PSUM") as psX_pool,
            tc.tile_pool(name="cep", bufs=1, space="PSUM") as cep_pool,
            tc.tile_pool(name="psC", bufs=1, space="PSUM") as psC_pool,
        ):
            bdc = consts.tile([128, 128], BF16, tag="bdc")
            nc.sync.dma_start(bdc[:], bdc_d[:])
            bds = consts.tile([128, 128], BF16, tag="bds")
            nc.sync.dma_start(bds[:], bds_d[:])
            ident = consts.tile([128, 128], BF16, tag="ident")
            nc.sync.dma_start(ident[:], id_d[:])
            idxt = consts.tile([128, 8], FP32, tag="idxt")
            nc.sync.dma_start(idxt[:], ix_d[:])
            identf = consts.tile([128, 128], FP32, tag="identf")
            nc.sync.dma_start(identf[:], idf_d[:])
            h2 = consts.tile([128, U * 3 * NV], BF16, tag="h2")
            nc.sync.dma_start(h2[:], h2_d[:].rearrange("s u c j -> s (u c j)"))
            ppj = consts.tile([128, 128], F32R, tag="ppj")
            nc.sync.dma_start(ppj[:], pp_d[:].rearrange("s p j -> s (p j)"))
            epsb = consts.tile([128, 1], FP32, tag="epsb")
            nc.vector.memset(epsb[:], 1e-10)
            symt = consts.tile([128, BLOC], I32, tag="symt")
            nc.sync.dma_start(symt[:], syms[:].rearrange("(c i) -> i c", i=128))

            def h2c(uu, comp):  # stationary slice for stage-2
                off = uu * (3 * NV) + comp * NV
                return h2[:, off:off + NV]

            for it in range(ITERS):
                xt = xt_pool.tile([128, WPI * 32], BF16, tag="xt")
                nc.sync.dma_start(
                    xt[:].rearrange("p (g s) -> p g s", s=S),
                    audio[it * WPI:(it + 1) * WPI, :]
                    .rearrange("(g w4) (t s) -> (w4 t) g s", w4=4, s=S))

                at_re = at_pool.tile([128, G * 128], BF16, tag="at_re")
                at_im = at_pool.tile([128, G * 128], BF16, tag="at_im")

                for gp in range(G // 2):   # 2 windows-groups (8 windows) per bank
                    psA = psA_pool.tile([128, 512], FP32, tag="psA")
                    x0 = xt[:, gp * 256:gp * 256 + 128]
                    x1 = xt[:, gp * 256 + 128:gp * 256 + 256]
                    nc.tensor.matmul(psA[:, 0:128], bdc[:], x0, start=True, stop=True)
                    nc.tensor.matmul(psA[:, 256:384], bdc[:], x1, start=True, stop=True)
                    nc.tensor.matmul(psA[:, 128:256], bds[:], x0, start=True, stop=True)
                    nc.tensor.matmul(psA[:, 384:512], bds[:], x1, start=True, stop=True)

                    asb = as_pool.tile([128, 512], BF16, tag="asb")
                    if gp % 2 == 0:
                        nc.vector.tensor_copy(asb[:], psA[:])
                    else:
                        nc.scalar.activation(asb[:], psA[:],
                                             mybir.ActivationFunctionType.Copy)

                    psT = psT_pool.tile([128, 512], BF16, tag="psT")
                    for j in range(4):
                        nc.tensor.transpose(psT[:, j * 128:(j + 1) * 128],
                                            asb[:, j * 128:(j + 1) * 128], ident[:])
                    # psT blocks: [Atre_g, Atim_g, Atre_g', Atim_g']
                    g0 = 2 * gp
                    dst_re = at_re[:, g0 * 128:(g0 + 2) * 128].rearrange(
                        "s (g c) -> s g c", g=2)
                    src_re = psT[:].rearrange("s (g x c) -> s g x c", g=2, x=2)[:, :, 0, :]
                    dst_im = at_im[:, g0 * 128:(g0 + 2) * 128].rearrange(
                        "s (g c) -> s g c", g=2)
                    src_im = psT[:].rearrange("s (g x c) -> s g x c", g=2, x=2)[:, :, 1, :]
                    if gp % 2 == 0:
                        nc.scalar.activation(dst_re, src_re,
                                             mybir.ActivationFunctionType.Copy)
                        nc.vector.tensor_copy(dst_im, src_im)
                    else:
                        nc.vector.tensor_copy(dst_re, src_re)
                        nc.scalar.activation(dst_im, src_im,
                                             mybir.ActivationFunctionType.Copy)

                cep = cep_pool.tile([128, 256], FP32, tag="cep")
                for p in range(16):
                    psX = psX_pool.tile([128, 512], FP32, tag="psX")
                    for half in range(2):
                        uu = 2 * p + half
                        ro = slice(half * 64, half * 64 + 64)
                        # moving operand: columns {g*128 + w4*32 + uu}
                        rre = at_re[:].rearrange("s (g w4 u) -> s g w4 u",
                                                 g=G, w4=4)[:, :, :, uu]
                        rim = at_im[:].rearrange("s (g w4 u) -> s g w4 u",
                                                 g=G, w4=4)[:, :, :, uu]
                        nc.tensor.matmul(psX[ro, 0:256], h2c(uu, 0), rre,
                                         start=True, stop=False)
                        nc.tensor.matmul(psX[ro, 0:256], h2c(uu, 2), rim,
                                         start=False, stop=True)
                        nc.tensor.matmul(psX[ro, 256:512], h2c(uu, 1), rre,
                                         start=True, stop=False)
                        nc.tensor.matmul(psX[ro, 256:512], h2c(uu, 0), rim,
                                         start=False, stop=True)
                    sq = sq_pool.tile([128, 512], FP32, tag="sq")
                    nc.scalar.activation(sq[:], psX[:],
                                         mybir.ActivationFunctionType.Square)
                    m2 = m2_pool.tile([128, 256], FP32, tag="m2")
                    nc.vector.tensor_add(m2[:], sq[:, 0:256], sq[:, 256:512])
                    lg = lg_pool.tile([128, 256], F32R, tag="lg")
                    nc.scalar.activation(lg[:], m2[:],
                                         mybir.ActivationFunctionType.Ln,
                                         bias=epsb[:])
                    nc.tensor.matmul(cep[0:8, :], ppj[:, p * 8:(p + 1) * 8], lg[:],
                                     start=(p == 0), stop=(p == 15))

                cep_sb = fin_pool.tile([8, 256], FP32, tag="cep_sb")
                nc.scalar.activation(cep_sb[:], cep[0:8, :],
                                     mybir.ActivationFunctionType.Copy)
                for c in range(2):
                    gc = it * 2 + c
                    psC = psC_pool.tile([128, 8], FP32, tag="psC")
                    nc.tensor.transpose(psC[:], cep_sb[:, c * 128:(c + 1) * 128],
                                        identf[0:8, 0:8])
                    mx = fin_pool.tile([128, 1], FP32, tag="mx")
                    nc.vector.reduce_max(mx[:], psC[:], axis=mybir.AxisListType.X)
                    nb = fin_pool.tile([128, 1], FP32, tag="nb")
                    nc.vector.tensor_scalar_mul(nb[:], mx[:], -BETA)
                    ex = fin_pool.tile([128, 8], FP32, tag="ex")
                    nc.scalar.activation(ex[:], psC[:],
                                         mybir.ActivationFunctionType.Exp,
                                         bias=nb[:], scale=BETA)
                    den = fin_pool.tile([128, 1], FP32, tag="den")
                    nc.vector.reduce_sum(den[:], ex[:], axis=mybir.AxisListType.X)
                    en = fin_pool.tile([128, 8], FP32, tag="en")
                    nc.vector.tensor_mul(en[:], ex[:], idxt[:])
                    num = fin_pool.tile([128, 1], FP32, tag="num")
                    nc.vector.reduce_sum(num[:], en[:], axis=mybir.AxisListType.X)
                    rden = fin_pool.tile([128, 1], FP32, tag="rden")
                    nc.vector.reciprocal(rden[:], den[:])
                    mv = fin_pool.tile([128, 1], FP32, tag="mv")
                    nc.vector.tensor_mul(mv[:], num[:], rden[:])
                    symf = fin_pool.tile([128, 1], FP32, tag="symf")
                    nc.vector.tensor_copy(symf[:], symt[:, gc:gc + 1])
                    df = fin_pool.tile([128, 1], FP32, tag="df")
                    nc.vector.tensor_sub(df[:], mv[:], symf[:])
                    ab = fin_pool.tile([128, 1], FP32, tag="ab")
                    nc.scalar.activation(ab[:], df[:],
                                         mybir.ActivationFunctionType.Abs)
                    ls = fin_pool.tile([128, 1], FP32, tag="ls")
                    nc.vector.tensor_scalar_min(ls[:], ab[:], 1.0)
                    nc.sync.dma_start(
                        loss_out[gc * 128:(gc + 1) * 128], ls[:, 0])
    return nc


def kernel(audio_batch, symbols_batch, num_errs_no_reverb_batch,
           num_errs_reverb_batch):
    audio_batch = np.asarray(audio_batch)
    symbols_batch = np.asarray(symbols_batch, dtype=np.int32)
    nn_ = np.asarray(num_errs_no_reverb_batch).astype(np.float32)
    nr_ = np.asarray(num_errs_reverb_batch).astype(np.float32)

    if "nc" not in _cache:
        _cache["nc"] = _install_hoist(_build())
        _cache["tabs"] = _tables()
    nc = _cache["nc"]
    bdc, bds, h2, pp, ident, idxt = _cache["tabs"]

    audio_bf = (audio_batch.reshape(B, NW * WIN)
                .astype(ml_dtypes.bfloat16)
                .reshape(NCORES, WLOC, WIN))
    syms = symbols_batch.reshape(NCORES, WLOC)
    in_maps = []
    for c in range(NCORES):
        in_maps.append({
            "audio": audio_bf[c], "syms": syms[c],
            "bdc": bdc, "bds": bds, "h2": h2, "pp": pp,
            "ident": ident, "idxt": idxt,
            "identf": np.asarray(ident, dtype=np.float32),
        })
    import os
    res = run_bass_kernel_spmd(nc, in_maps, core_ids=list(range(NCORES)),
                               trace=bool(os.environ.get("KTRACE")))
    _cache["last_res"] = res
    loss = np.concatenate([res.results[c]["loss_out"] for c in range(NCORES)])
    errs = loss.reshape(B, NW).sum(axis=1, dtype=np.float32)

    tot = np.float32(errs.sum())
    diff = nr_ - nn_
    inv_red = np.where(diff == 0, np.float32(1.0), diff / (nr_ - errs))
    ter = np.float32(inv_red.sum())
    denom = np.float32(B * NW)
    return (np.float32(tot / denom), tot, np.float32(ter / B),
            np.float32(nn_.sum() / denom), np.float32(nr_.sum() / denom))



# revision 4
# speedup vs baseline: 2.0458x; 2.0458x over previous
"""Trainium2 Bass kernel for nn_DecodingLoss (cepstrum decoding loss).

Math (per 4096-sample window):
  cep = irfft(log(|rfft(x)| + eps))[DELAYS]; softargmax(beta=1e10) ~= hard argmax;
  loss = clip(|idx - symbol|,0,1) = 1[argmax != symbol]; per-audio sums -> 5 scalars.

Kernel strategy (8 cores, pure data parallel over the batch dim; 1024 windows/core):
  FFT 4096 = 32 x 128 Cooley-Tukey, n = 128 t + s (t<32, s<128), k = u + 32 v.
  stage1 (PE): per 4-window group, stationary = x4 [(w4 t), s], moving = block-diag
    W32 table -> psA = A^T[s, (w4, u-re/im)] directly (no transpose step). Real input
    hermitian symmetry: only u=0..16 kept (u0/u16 real), 32 cols per window.
  stage2 (PE): for q=1..15 pair k-sets {q+32v} and {32-q+32v} (conjugate u's) share
    the same moving operands rre/rim; 128-wide stationaries put Re of both sets in
    psX[:,0:256] and Im in psX[:,256:512] -> |X|^2 = one aligned full-width add.
    u=0/16 handled via a PE stacked-identity sum.
  log|X|: ACT Square(scale 2^-6) -> bf16, DVE add, ACT Ln (values centered near 0 so
  bf16 is safe), then per-pair bf16 projection matmul accumulates cep[8, 256 win].
  Loss: transpose cep to [win, tap], then batched: sel = cep[sym] via one-hot mult,
  loss = min((max - sel)*1e12, 1). Host sums per-audio errors + final scalar math.
"""
import numpy as np
import ml_dtypes

import concourse.bass as bass
import concourse.mybir as mybir
from concourse import tile
from concourse.bass_utils import run_bass_kernel_spmd

FP32 = mybir.dt.float32
BF16 = mybir.dt.bfloat16
I32 = mybir.dt.int32

B, NW, WIN = 64, 128, 4096
NCORES = 8
BLOC = B // NCORES              # 8 audio rows per core
WLOC = BLOC * NW                # 1024 windows per core
T, S = 32, 128                  # n = 128 t + s
NV = 64                         # v-grid size per k-set
ITERS = 4
WPI = WLOC // ITERS             # 256 windows per iteration
G = WPI // 4                    # 64 groups of 4 windows
DELAYS = np.array([64, 96, 128, 160, 192, 224, 256, 288])
SQ_SCALE = 2.0 ** -6            # |X|^2 scaled by 2^-12: ln output centered near 0
LN_EPS = 2.44e-14

_cache = {}


def _hoist_waits(bir_json):
    """This walrus build rejects instructions carrying attached semaphore waits
    ("Too many sync wait commands"); raw-bass style standalone EventSemaphore
    waits compile and run. Hoist every attached wait into its own
    EventSemaphore on the same engine queue; updates stay attached."""
    import json
    d = json.loads(bir_json)
    n = 0
    for fn in d["functions"]:
        for bb in fn["blocks"]:
            out = []
            for ins in bb["instructions"]:
                si = ins.get("sync_info")
                waits = (si or {}).get("on_wait") or []
                if waits and ins.get("opcode") != "EventSemaphore" and ins.get("engine"):
                    for w in waits:
                        n += 1
                        out.append({
                            "name": f"hoistw-{n}", "opcode": "EventSemaphore",
                            "engine": ins["engine"], "ins": [], "outs": [],
                            "sync_info": {"on_wait": [w], "on_update": []},
                        })
                    si["on_wait"] = []
                out.append(ins)
            bb["instructions"] = out
    return json.dumps(d).encode()


def _install_hoist(nc):
    orig = nc.to_json_bytes
    nc.to_json_bytes = lambda: _hoist_waits(orig())
    return nc


def _tables():
    t = np.arange(T)
    # BDCS [128,128]: rows (w4,t), cols (w4,jj); jj: 0=re u0, 1=re u16,
    # 2..16=re u=1..15, 17..31=im u=1..15
    blk = np.zeros((32, 32))
    blk[:, 0] = 1.0
    blk[:, 1] = np.cos(np.pi * t)
    for u in range(1, 16):
        blk[:, u + 1] = np.cos(2 * np.pi * t * u / 32.0)
        blk[:, u + 16] = -np.sin(2 * np.pi * t * u / 32.0)
    bdcs = np.zeros((128, 128))
    for w in range(4):
        bdcs[w * 32:(w + 1) * 32, w * 32:(w + 1) * 32] = blk

    s = np.arange(S)[:, None]
    v = np.arange(NV)[None, :]
    # ss [128, 62*128]: q=1..15 -> blocks (q-1)*4 + {SR1,SR2,SI1,SI2}; S0=60, S16=61
    ss = np.zeros((128, 62 * 128))
    for q in range(1, 16):
        phA = 2 * np.pi * s * (q + 32 * v) / 4096.0
        phB = 2 * np.pi * s * ((32 - q) + 32 * v) / 4096.0
        o = (q - 1) * 4 * 128
        ss[:, o:o + 128] = np.hstack([np.cos(phA), np.cos(phB)])       # SR1 @ rre
        ss[:, o + 128:o + 256] = np.hstack([np.sin(phA), -np.sin(phB)])  # SR2 @ rim
        ss[:, o + 256:o + 384] = np.hstack([-np.sin(phA), -np.sin(phB)])  # SI1 @ rre
        ss[:, o + 384:o + 512] = np.hstack([np.cos(phA), -np.cos(phB)])  # SI2 @ rim
    ph0 = 2 * np.pi * s * (32 * (v + 1)) / 4096.0
    ph16 = 2 * np.pi * s * (16 + 32 * v) / 4096.0
    ss[:, 60 * 128:61 * 128] = np.hstack([np.cos(ph0), -np.sin(ph0)])
    ss[:, 61 * 128:62 * 128] = np.hstack([np.cos(ph16), -np.sin(ph16)])

    vv = np.arange(NV)

    def ppcol(k):  # [64, 8]
        wk = np.where(k == 2048, 1.0, 2.0)
        return (wk[:, None] * 0.5 *
                np.cos(2 * np.pi * k[:, None] * DELAYS[None, :] / 4096.0) / 4096.0)

    ppj = np.zeros((128, 15 * 8))
    for q in range(1, 16):
        ppj[0:64, (q - 1) * 8:q * 8] = ppcol(q + 32 * vv)
        ppj[64:128, (q - 1) * 8:q * 8] = ppcol((32 - q) + 32 * vv)
    pp016 = np.zeros((64, 16))
    pp016[:, 0:8] = ppcol(32 * (vv + 1))
    pp016[:, 8:16] = ppcol(16 + 32 * vv)

    i64b = np.zeros((128, 64))
    i64b[np.arange(128), np.arange(128) % 64] = 1.0
    ident8 = np.eye(8)
    bf = ml_dtypes.bfloat16
    return (bdcs.astype(bf), ss.astype(bf), ppj.astype(bf), pp016.astype(bf),
            i64b.astype(bf), ident8.astype(np.float32))


def _build():
    nc = bass.Bass()
    audio = nc.dram_tensor("audio", [ITERS * 128, G * 128], BF16, kind="ExternalInput")
    bdcs_d = nc.dram_tensor("bdcs", [128, 128], BF16, kind="ExternalInput")
    ss_d = nc.dram_tensor("ss", [128, 62 * 128], BF16, kind="ExternalInput")
    ppj_d = nc.dram_tensor("ppj", [128, 120], BF16, kind="ExternalInput")
    pp016_d = nc.dram_tensor("pp016", [64, 16], BF16, kind="ExternalInput")
    oh_d = nc.dram_tensor("onehot", [128, 64], FP32, kind="ExternalInput")
    i64_d = nc.dram_tensor("i64b", [128, 64], BF16, kind="ExternalInput")
    id8_d = nc.dram_tensor("ident8", [8, 8], FP32, kind="ExternalInput")
    loss_out = nc.dram_tensor("loss_out", [128, 8], FP32, kind="ExternalOutput")

    with tile.TileContext(nc) as tc:
        with (
            tc.tile_pool(name="consts", bufs=1) as consts,
            tc.tile_pool(name="xt", bufs=2) as xt_pool,
            tc.tile_pool(name="at", bufs=2) as at_pool,
            tc.tile_pool(name="sq", bufs=2) as sq_pool,
            tc.tile_pool(name="m2", bufs=2) as m2_pool,
            tc.tile_pool(name="lg", bufs=4) as lg_pool,
            tc.tile_pool(name="lg0", bufs=2) as lg0_pool,
            tc.tile_pool(name="fin", bufs=2) as fin_pool,
            tc.tile_pool(name="psA", bufs=2, space="PSUM") as psA_pool,
            tc.tile_pool(name="psX", bufs=2, space="PSUM") as psX_pool,
            tc.tile_pool(name="psM", bufs=1, space="PSUM") as psM_pool,
            tc.tile_pool(name="cep", bufs=2, space="PSUM") as cep_pool,
            tc.tile_pool(name="psC", bufs=1, space="PSUM") as psC_pool,
        ):
            bdcs = consts.tile([128, 128], BF16, tag="bdcs")
            nc.sync.dma_start(bdcs[:], bdcs_d[:])
            ss = consts.tile([128, 62 * 128], BF16, tag="ss")
            nc.sync.dma_start(ss[:], ss_d[:])
            ppj = consts.tile([128, 120], BF16, tag="ppj")
            nc.sync.dma_start(ppj[:], ppj_d[:])
            pp016 = consts.tile([64, 16], BF16, tag="pp016")
            nc.sync.dma_start(pp016[:], pp016_d[:])
            onehot = consts.tile([128, 64], FP32, tag="onehot")
            nc.sync.dma_start(onehot[:], oh_d[:])
            i64b = consts.tile([128, 64], BF16, tag="i64b")
            nc.sync.dma_start(i64b[:], i64_d[:])
            ident8 = consts.tile([8, 8], FP32, tag="ident8")
            nc.sync.dma_start(ident8[:], id8_d[:])
            epsb = consts.tile([128, 1], FP32, tag="epsb")
            nc.vector.memset(epsb[:], LN_EPS)
            cepT = consts.tile([128, 64], FP32, tag="cepT")

            def sblk(b):  # stationary block b of ss
                return ss[:, b * 128:(b + 1) * 128]

            pend = []   # deferred cep-finalize emitters from previous iteration

            for it in range(ITERS):
                xt = xt_pool.tile([128, G * 128], BF16, tag="xt")
                for c in range(4):
                    nc.sync.dma_start(
                        xt[:, c * 2048:(c + 1) * 2048],
                        audio[it * 128:(it + 1) * 128, c * 2048:(c + 1) * 2048])
                at = at_pool.tile([128, G * 128], BF16, tag="at")

                for g0 in range(0, G, 4):
                    psA = psA_pool.tile([128, 512], FP32, tag="psA")
                    for g in range(g0, g0 + 4):
                        nc.tensor.matmul(psA[:, (g - g0) * 128:(g - g0 + 1) * 128],
                                         xt[:, g * 128:(g + 1) * 128], bdcs[:],
                                         start=True, stop=True)
                    if g0 == 4:
                        for fn in pend:
                            fn()
                        pend = []
                    dst = at[:, g0 * 128:(g0 + 4) * 128]
                    if g0 % 8 == 0:
                        nc.vector.tensor_copy(dst, psA[:])
                    else:
                        nc.scalar.activation(dst, psA[:],
                                             mybir.ActivationFunctionType.Copy)

                atv = at[:].rearrange("s (g w4 j) -> s g w4 j", w4=4, j=32)

                cep = cep_pool.tile([128, 256], FP32, tag="cep")
                projq = []   # delayed projection emitters
                nproj = 17
                emitted = [0]

                def emit_proj():
                    fn = projq.pop(0)
                    fn()
                    emitted[0] += 1

                def mk_proj(stat, lgt):
                    def fn():
                        nc.tensor.matmul(cep[0:8, :], stat, lgt,
                                         start=(emitted[0] == 0),
                                         stop=(emitted[0] == nproj - 1))
                    return fn

                lg0 = None
                for q in range(1, 16):
                    rre = atv[:, :, :, q + 1]
                    rim = atv[:, :, :, q + 16]
                    o = (q - 1) * 4
                    psX = psX_pool.tile([128, 512], FP32, tag="psX")
                    nc.tensor.matmul(psX[:, 0:256], sblk(o), rre, start=True, stop=False)
                    nc.tensor.matmul(psX[:, 0:256], sblk(o + 1), rim, start=False, stop=True)
                    nc.tensor.matmul(psX[:, 256:512], sblk(o + 2), rre, start=True, stop=False)
                    nc.tensor.matmul(psX[:, 256:512], sblk(o + 3), rim, start=False, stop=True)
                    sq = sq_pool.tile([128, 512], BF16, tag="sq")
                    nc.scalar.activation(sq[:], psX[:],
                                         mybir.ActivationFunctionType.Square,
                                         scale=SQ_SCALE)
                    m2 = m2_pool.tile([128, 256], BF16, tag="m2")
                    nc.vector.tensor_add(m2[:], sq[:, 0:256], sq[:, 256:512])
                    lg = lg_pool.tile([128, 256], BF16, tag="lg")
                    nc.scalar.activation(lg[:], m2[:], mybir.ActivationFunctionType.Ln,
                                         bias=epsb[:])
                    projq.append(mk_proj(ppj[:, (q - 1) * 8:q * 8], lg[:]))

                    if q == 2:
                        # u=0/16 singleton: psX0 col-split re/im, PE-sum via i64b
                        psX0 = psX_pool.tile([128, 512], FP32, tag="psX")
                        nc.tensor.matmul(psX0[:, 0:256], sblk(60), atv[:, :, :, 0],
                                         start=True, stop=True)
                        nc.tensor.matmul(psX0[:, 256:512], sblk(61), atv[:, :, :, 1],
                                         start=True, stop=True)
                        sq0 = sq_pool.tile([128, 512], BF16, tag="sq")
                        nc.scalar.activation(sq0[:], psX0[:],
                                             mybir.ActivationFunctionType.Square,
                                             scale=SQ_SCALE)
                    if q == 4:
                        psM = psM_pool.tile([64, 512], FP32, tag="psM")
                        nc.tensor.matmul(psM[:], i64b[:], sq0[:], start=True, stop=True)
                        lg0 = lg0_pool.tile([64, 512], BF16, tag="lg0")
                        nc.scalar.activation(lg0[:], psM[:],
                                             mybir.ActivationFunctionType.Ln,
                                             bias=epsb[0:64])
                        projq.append(mk_proj(pp016[:, 0:8], lg0[:, 0:256]))
                        projq.append(mk_proj(pp016[:, 8:16], lg0[:, 256:512]))

                    if q >= 3:
                        emit_proj()
                while projq:
                    emit_proj()

                def mk_fin(it, cep):
                    def fn():
                        cep_sb = fin_pool.tile([8, 256], FP32, tag="cep_sb")
                        nc.scalar.activation(cep_sb[:], cep[0:8, :],
                                             mybir.ActivationFunctionType.Copy)
                        for c in range(2):
                            gc = it * 2 + c
                            psC = psC_pool.tile([128, 8], FP32, tag="psC")
                            nc.tensor.transpose(psC[:], cep_sb[:, c * 128:(c + 1) * 128],
                                                ident8[:])
                            nc.vector.tensor_copy(cepT[:, gc * 8:(gc + 1) * 8], psC[:])
                    return fn

                if it < ITERS - 1:
                    pend.append(mk_fin(it, cep))
                else:
                    mk_fin(it, cep)()

            # batched loss over all 1024 windows: [128 w, 8 audios]
            tmp = fin_pool.tile([128, 64], FP32, tag="tmp")
            nc.vector.tensor_mul(tmp[:], cepT[:], onehot[:])
            sel = fin_pool.tile([128, 8], FP32, tag="sel")
            nc.vector.reduce_sum(sel[:], tmp[:].rearrange("p (a j) -> p a j", j=8),
                                 axis=mybir.AxisListType.X)
            mx = fin_pool.tile([128, 8], FP32, tag="mx")
            nc.vector.reduce_max(mx[:], cepT[:].rearrange("p (a j) -> p a j", j=8),
                                 axis=mybir.AxisListType.X)
            df = fin_pool.tile([128, 8], FP32, tag="df")
            nc.vector.tensor_sub(df[:], mx[:], sel[:])
            df2 = fin_pool.tile([128, 8], FP32, tag="df2")
            nc.vector.tensor_scalar_mul(df2[:], df[:], 1e12)
            ls = fin_pool.tile([128, 8], FP32, tag="ls")
            nc.vector.tensor_scalar_min(ls[:], df2[:], 1.0)
            nc.sync.dma_start(loss_out[:], ls[:])
    return nc


def kernel(audio_batch, symbols_batch, num_errs_no_reverb_batch,
           num_errs_reverb_batch):
    audio_batch = np.asarray(audio_batch)
    symbols_batch = np.asarray(symbols_batch, dtype=np.int32)
    nn_ = np.asarray(num_errs_no_reverb_batch).astype(np.float32)
    nr_ = np.asarray(num_errs_reverb_batch).astype(np.float32)

    if "nc" not in _cache:
        _cache["nc"] = _install_hoist(_build())
        _cache["tabs"] = _tables()
    nc = _cache["nc"]
    bdcs, ss, ppj, pp016, i64b, ident8 = _cache["tabs"]

    # host pre-transpose: [core][it, (w4 t), (g s)] so device DMA is contiguous
    wins = (audio_batch.reshape(NCORES, WLOC, T, S)
            .reshape(NCORES, ITERS, G, 4, T, S)
            .transpose(0, 1, 3, 4, 2, 5)
            .reshape(NCORES, ITERS * 128, G * 128)
            .astype(ml_dtypes.bfloat16))
    sy = symbols_batch.reshape(NCORES, BLOC, NW)
    in_maps = []
    for c in range(NCORES):
        oh = (sy[c].T[:, :, None] == np.arange(8)).astype(np.float32).reshape(128, 64)
        in_maps.append({
            "audio": wins[c], "onehot": oh,
            "bdcs": bdcs, "ss": ss, "ppj": ppj, "pp016": pp016,
            "i64b": i64b, "ident8": ident8,
        })
    import os
    res = run_bass_kernel_spmd(nc, in_maps, core_ids=list(range(NCORES)),
                               trace=bool(os.environ.get("KTRACE")))
    _cache["last_res"] = res
    errs = np.zeros(B, np.float32)
    for c in range(NCORES):
        loss = res.results[c]["loss_out"]          # [128 w, 8 audios]
        errs[c * BLOC:(c + 1) * BLOC] = loss.sum(axis=0, dtype=np.float32)

    tot = np.float32(errs.sum())
    diff = nr_ - nn_
    inv_red = np.where(diff == 0, np.float32(1.0), diff / (nr_ - errs))
    ter = np.float32(inv_red.sum())
    denom = np.float32(B * NW)
    return (np.float32(tot / denom), tot, np.float32(ter / B),
            np.float32(nn_.sum() / denom), np.float32(nr_.sum() / denom))


# revision 12
# speedup vs baseline: 3.1995x; 1.5639x over previous
"""Trainium2 Bass kernel for nn_DecodingLoss (cepstrum decoding loss).

Math (per 4096-sample window):
  cep = irfft(log(|rfft(x)| + eps))[DELAYS]; softargmax(beta=1e10) ~= hard argmax;
  loss = clip(|idx - symbol|,0,1) = 1[argmax != symbol]; per-audio sums -> 5 scalars.

Kernel strategy (8 cores, pure data parallel over the batch dim; 1024 windows/core):
  FFT 4096 = 32 x 128 Cooley-Tukey, n = 128 t + s (t<32, s<128), k = u + 32 v.
  stage1 (PE): per 4-window group, stationary = x4 [(w4 t), s], moving = block-diag
    W32 table -> psA = A^T[s, (w4, u-re/im)] directly (no transpose step). Real input
    hermitian symmetry: only u=0..16 kept (u0/u16 real), 32 cols per window.
  stage2 (PE): for q=1..15 pair k-sets {q+32v} and {32-q+32v} (conjugate u's) share
    the same moving operands rre/rim; 128-wide stationaries put Re of both sets in
    psX[:,0:256] and Im in psX[:,256:512] -> |X|^2 = one aligned full-width add.
    u=0/16 handled via a PE stacked-identity sum.
  log|X|: ACT Square(scale 2^-6) -> bf16, DVE add, ACT Ln (values centered near 0 so
  bf16 is safe), then per-pair bf16 projection matmul accumulates cep[8, 256 win].
  Loss: transpose cep to [win, tap], then batched: sel = cep[sym] via one-hot mult,
  loss = min((max - sel)*1e12, 1). Host sums per-audio errors + final scalar math.
"""
import numpy as np
import ml_dtypes

import concourse.bass as bass
import concourse.mybir as mybir
from concourse import tile
from concourse.bass_utils import run_bass_kernel_spmd

FP32 = mybir.dt.float32
BF16 = mybir.dt.bfloat16
I32 = mybir.dt.int32

B, NW, WIN = 64, 128, 4096
NCORES = 8
BLOC = B // NCORES              # 8 audio rows per core
WLOC = BLOC * NW                # 1024 windows per core
T, S = 32, 128                  # n = 128 t + s
NV = 64                         # v-grid size per k-set
ITERS = 4
WPI = WLOC // ITERS             # 256 windows per iteration
G = WPI // 4                    # 64 groups of 4 windows
DELAYS = np.array([64, 96, 128, 160, 192, 224, 256, 288])
SQ_SCALE = 2.0 ** -6            # |X|^2 scaled by 2^-12: ln output centered near 0
LN_EPS = 2.44e-14

_cache = {}


def _hoist_waits(bir_json):
    """This walrus build rejects instructions carrying attached semaphore waits
    ("Too many sync wait commands"); raw-bass style standalone EventSemaphore
    waits compile and run. Hoist every attached wait into its own
    EventSemaphore on the same engine queue; updates stay attached."""
    import json
    d = json.loads(bir_json)
    n = 0
    for fn in d["functions"]:
        for bb in fn["blocks"]:
            out = []
            for ins in bb["instructions"]:
                si = ins.get("sync_info")
                waits = (si or {}).get("on_wait") or []
                if waits and ins.get("opcode") != "EventSemaphore" and ins.get("engine"):
                    for w in waits:
                        n += 1
                        out.append({
                            "name": f"hoistw-{n}", "opcode": "EventSemaphore",
                            "engine": ins["engine"], "ins": [], "outs": [],
                            "sync_info": {"on_wait": [w], "on_update": []},
                        })
                    si["on_wait"] = []
                out.append(ins)
            bb["instructions"] = out
    return json.dumps(d).encode()


def _install_hoist(nc):
    orig = nc.to_json_bytes
    nc.to_json_bytes = lambda: _hoist_waits(orig())
    return nc


def _tables():
    t = np.arange(T)
    # BDCS [128,128]: rows (w4,t), cols (w4,jj); jj: 0=re u0, 1=re u16,
    # 2..16=re u=1..15, 17..31=im u=1..15
    blk = np.zeros((32, 32))
    blk[:, 0] = 1.0
    blk[:, 1] = np.cos(np.pi * t)
    for u in range(1, 16):
        blk[:, u + 1] = np.cos(2 * np.pi * t * u / 32.0)
        blk[:, u + 16] = -np.sin(2 * np.pi * t * u / 32.0)
    bdcs = np.zeros((128, 128))
    for w in range(4):
        bdcs[w * 32:(w + 1) * 32, w * 32:(w + 1) * 32] = blk
    # stage1 psA emits (jj, w4) column order so the at-copy scatters to j-major
    perm = np.array([w4 * 32 + jj for jj in range(32) for w4 in range(4)])
    bdcs = bdcs[:, perm]

    s = np.arange(S)[:, None]
    v = np.arange(NV)[None, :]
    # ss [128, 62*128]: q=1..15 -> blocks (q-1)*4 + {SR1,SR2,SI1,SI2}; S0=60, S16=61
    ss = np.zeros((128, 62 * 128))
    for q in range(1, 16):
        phA = 2 * np.pi * s * (q + 32 * v) / 4096.0
        phB = 2 * np.pi * s * ((32 - q) + 32 * v) / 4096.0
        o = (q - 1) * 4 * 128
        ss[:, o:o + 128] = np.hstack([np.cos(phA), np.cos(phB)])       # SR1 @ rre
        ss[:, o + 128:o + 256] = np.hstack([np.sin(phA), -np.sin(phB)])  # SR2 @ rim
        ss[:, o + 256:o + 384] = np.hstack([-np.sin(phA), -np.sin(phB)])  # SI1 @ rre
        ss[:, o + 384:o + 512] = np.hstack([np.cos(phA), -np.cos(phB)])  # SI2 @ rim
    ph0 = 2 * np.pi * s * (32 * (v + 1)) / 4096.0
    ph16 = 2 * np.pi * s * (16 + 32 * v) / 4096.0
    ss[:, 60 * 128:61 * 128] = np.hstack([np.cos(ph0), -np.sin(ph0)])
    ss[:, 61 * 128:62 * 128] = np.hstack([np.cos(ph16), -np.sin(ph16)])

    vv = np.arange(NV)

    def ppcol(k):  # [64, 8]
        wk = np.where(k == 2048, 1.0, 2.0)
        return (wk[:, None] * 0.5 *
                np.cos(2 * np.pi * k[:, None] * DELAYS[None, :] / 4096.0) / 4096.0)

    ppj = np.zeros((128, 15 * 8))
    for q in range(1, 16):
        ppj[0:64, (q - 1) * 8:q * 8] = ppcol(q + 32 * vv)
        ppj[64:128, (q - 1) * 8:q * 8] = ppcol((32 - q) + 32 * vv)
    pp016 = np.zeros((64, 16))
    pp016[:, 0:8] = ppcol(32 * (vv + 1))
    pp016[:, 8:16] = ppcol(16 + 32 * vv)

    i64b = np.zeros((128, 64))
    i64b[np.arange(128), np.arange(128) % 64] = 1.0
    ident8 = np.eye(8)
    bf = ml_dtypes.bfloat16
    return (bdcs.astype(bf), ss.astype(bf), ppj.astype(bf), pp016.astype(bf),
            i64b.astype(bf), ident8.astype(np.float32))


def _build():
    nc = bass.Bass()
    audio = nc.dram_tensor("audio", [ITERS * 128, G * 128], BF16, kind="ExternalInput")
    bdcs_d = nc.dram_tensor("bdcs", [128, 128], BF16, kind="ExternalInput")
    ss_d = nc.dram_tensor("ss", [128, 62 * 128], BF16, kind="ExternalInput")
    ppj_d = nc.dram_tensor("ppj", [128, 120], BF16, kind="ExternalInput")
    pp016_d = nc.dram_tensor("pp016", [64, 16], BF16, kind="ExternalInput")
    oh_d = nc.dram_tensor("onehot", [128, 64], FP32, kind="ExternalInput")
    i64_d = nc.dram_tensor("i64b", [128, 64], BF16, kind="ExternalInput")
    id8_d = nc.dram_tensor("ident8", [8, 8], FP32, kind="ExternalInput")
    loss_out = nc.dram_tensor("loss_out", [128, 8], FP32, kind="ExternalOutput")

    with tile.TileContext(nc) as tc:
        with (
            tc.tile_pool(name="consts", bufs=1) as consts,
            tc.tile_pool(name="xt", bufs=2) as xt_pool,
            tc.tile_pool(name="at", bufs=2) as at_pool,
            tc.tile_pool(name="sq", bufs=2) as sq_pool,
            tc.tile_pool(name="m2", bufs=2) as m2_pool,
            tc.tile_pool(name="lg", bufs=4) as lg_pool,
            tc.tile_pool(name="lg0", bufs=2) as lg0_pool,
            tc.tile_pool(name="fin", bufs=2) as fin_pool,
            tc.tile_pool(name="psA", bufs=3, space="PSUM") as psA_pool,
            tc.tile_pool(name="psX", bufs=2, space="PSUM") as psX_pool,
            tc.tile_pool(name="cep", bufs=2, space="PSUM") as cep_pool,
            tc.tile_pool(name="psC", bufs=1, space="PSUM") as psC_pool,
        ):
            bdcs = consts.tile([128, 128], BF16, tag="bdcs")
            nc.sync.dma_start(bdcs[:], bdcs_d[:])
            ss = consts.tile([128, 62 * 128], BF16, tag="ss")
            nc.sync.dma_start(ss[:], ss_d[:])
            ppj = consts.tile([128, 120], BF16, tag="ppj")
            nc.sync.dma_start(ppj[:], ppj_d[:])
            pp016 = consts.tile([64, 16], BF16, tag="pp016")
            nc.sync.dma_start(pp016[:], pp016_d[:])
            onehot = consts.tile([128, 64], FP32, tag="onehot")
            nc.sync.dma_start(onehot[:], oh_d[:])
            i64b = consts.tile([128, 64], BF16, tag="i64b")
            nc.sync.dma_start(i64b[:], i64_d[:])
            ident8 = consts.tile([8, 8], FP32, tag="ident8")
            nc.sync.dma_start(ident8[:], id8_d[:])
            epsb = consts.tile([128, 1], FP32, tag="epsb")
            nc.vector.memset(epsb[:], LN_EPS)
            cepT = consts.tile([128, 64], FP32, tag="cepT")

            def sblk(b):  # stationary block b of ss
                return ss[:, b * 128:(b + 1) * 128]

            pend = []   # deferred cep-finalize emitters from previous iteration

            for it in range(ITERS):
                xt = xt_pool.tile([128, G * 128], BF16, tag="xt")
                for c in range(4):
                    nc.sync.dma_start(
                        xt[:, c * 2048:(c + 1) * 2048],
                        audio[it * 128:(it + 1) * 128, c * 2048:(c + 1) * 2048])
                at = at_pool.tile([128, G * 128], BF16, tag="at")

                # at is j-major: col = j*256 + w  (w = g*4 + w4), so stage2
                # moving operands are contiguous 256-col slices per j
                atv_j = at[:].rearrange("s (j g w4) -> s j g w4", j=32, w4=4)
                for g0 in range(0, G, 4):
                    psA = psA_pool.tile([128, 512], FP32, tag="psA")
                    for g in range(g0, g0 + 4):
                        nc.tensor.matmul(psA[:, (g - g0) * 128:(g - g0 + 1) * 128],
                                         xt[:, g * 128:(g + 1) * 128], bdcs[:],
                                         start=True, stop=True)
                    if g0 == 4:
                        for fn in pend:
                            fn()
                        pend = []
                    dst = atv_j[:, :, g0:g0 + 4, :]
                    src = psA[:].rearrange("s (g j w4) -> s j g w4", g=4, w4=4)
                    if g0 % 8 == 0:
                        nc.vector.tensor_copy(dst, src)
                    else:
                        nc.scalar.activation(dst, src,
                                             mybir.ActivationFunctionType.Copy)

                cep = cep_pool.tile([128, 256], FP32, tag="cep")
                projq = []   # delayed projection emitters
                nproj = 17
                emitted = [0]

                def emit_proj():
                    fn = projq.pop(0)
                    fn()
                    emitted[0] += 1

                def mk_proj(stat, lgt):
                    def fn():
                        nc.tensor.matmul(cep[0:8, :], stat, lgt,
                                         start=(emitted[0] == 0),
                                         stop=(emitted[0] == nproj - 1))
                    return fn

                lg0 = None
                for q in range(1, 16):
                    rre = at[:, (q + 1) * 256:(q + 2) * 256]
                    rim = at[:, (q + 16) * 256:(q + 17) * 256]
                    o = (q - 1) * 4
                    psX = psX_pool.tile([128, 512], FP32, tag="psX")
                    nc.tensor.matmul(psX[:, 0:256], sblk(o), rre, start=True, stop=False)
                    nc.tensor.matmul(psX[:, 0:256], sblk(o + 1), rim, start=False, stop=True)
                    nc.tensor.matmul(psX[:, 256:512], sblk(o + 2), rre, start=True, stop=False)
                    nc.tensor.matmul(psX[:, 256:512], sblk(o + 3), rim, start=False, stop=True)
                    sq = sq_pool.tile([128, 512], BF16, tag="sq")
                    nc.scalar.activation(sq[:], psX[:],
                                         mybir.ActivationFunctionType.Square,
                                         scale=SQ_SCALE)
                    m2 = m2_pool.tile([128, 256], BF16, tag="m2")
                    nc.gpsimd.tensor_add(m2[:], sq[:, 0:256], sq[:, 256:512])
                    lg = lg_pool.tile([128, 256], BF16, tag="lg")
                    nc.scalar.activation(lg[:], m2[:], mybir.ActivationFunctionType.Ln,
                                         bias=epsb[:])
                    projq.append(mk_proj(ppj[:, (q - 1) * 8:q * 8], lg[:]))

                    if q == 2:
                        # u=0/16 singleton: psX0 col-split re/im, PE-sum via i64b
                        psX0 = psX_pool.tile([128, 512], FP32, tag="psX")
                        nc.tensor.matmul(psX0[:, 0:256], sblk(60), at[:, 0:256],
                                         start=True, stop=True)
                        nc.tensor.matmul(psX0[:, 256:512], sblk(61), at[:, 256:512],
                                         start=True, stop=True)
                        sq0 = sq_pool.tile([128, 512], BF16, tag="sq")
                        nc.scalar.activation(sq0[:], psX0[:],
                                             mybir.ActivationFunctionType.Square,
                                             scale=SQ_SCALE)
                    if q == 4:
                        psM = psX_pool.tile([128, 512], FP32, tag="psX")
                        nc.tensor.matmul(psM[0:64, :], i64b[:], sq0[:],
                                         start=True, stop=True)
                        lg0 = lg0_pool.tile([64, 512], BF16, tag="lg0")
                        nc.scalar.activation(lg0[:], psM[0:64, :],
                                             mybir.ActivationFunctionType.Ln,
                                             bias=epsb[0:64])
                        projq.append(mk_proj(pp016[:, 0:8], lg0[:, 0:256]))
                        projq.append(mk_proj(pp016[:, 8:16], lg0[:, 256:512]))

                    if q >= 3:
                        emit_proj()
                while projq:
                    emit_proj()

                def mk_fin(it, cep):
                    def fn():
                        cep_sb = fin_pool.tile([8, 256], FP32, tag="cep_sb")
                        nc.scalar.activation(cep_sb[:], cep[0:8, :],
                                             mybir.ActivationFunctionType.Copy)
                        for c in range(2):
                            gc = it * 2 + c
                            psC = psC_pool.tile([128, 8], FP32, tag="psC")
                            nc.tensor.transpose(psC[:], cep_sb[:, c * 128:(c + 1) * 128],
                                                ident8[:])
                            nc.vector.tensor_copy(cepT[:, gc * 8:(gc + 1) * 8], psC[:])
                    return fn

                if it < ITERS - 1:
                    pend.append(mk_fin(it, cep))
                else:
                    mk_fin(it, cep)()

            # batched loss over all 1024 windows: [128 w, 8 audios]
            tmp = fin_pool.tile([128, 64], FP32, tag="tmp")
            nc.vector.tensor_mul(tmp[:], cepT[:], onehot[:])
            sel = fin_pool.tile([128, 8], FP32, tag="sel")
            nc.vector.reduce_sum(sel[:], tmp[:].rearrange("p (a j) -> p a j", j=8),
                                 axis=mybir.AxisListType.X)
            mx = fin_pool.tile([128, 8], FP32, tag="mx")
            nc.vector.reduce_max(mx[:], cepT[:].rearrange("p (a j) -> p a j", j=8),
                                 axis=mybir.AxisListType.X)
            df = fin_pool.tile([128, 8], FP32, tag="df")
            nc.vector.tensor_sub(df[:], mx[:], sel[:])
            df2 = fin_pool.tile([128, 8], FP32, tag="df2")
            nc.vector.tensor_scalar_mul(df2[:], df[:], 1e12)
            ls = fin_pool.tile([128, 8], FP32, tag="ls")
            nc.vector.tensor_scalar_min(ls[:], df2[:], 1.0)
            nc.sync.dma_start(loss_out[:], ls[:])
    return nc


def kernel(audio_batch, symbols_batch, num_errs_no_reverb_batch,
           num_errs_reverb_batch):
    audio_batch = np.asarray(audio_batch)
    symbols_batch = np.asarray(symbols_batch, dtype=np.int32)
    nn_ = np.asarray(num_errs_no_reverb_batch).astype(np.float32)
    nr_ = np.asarray(num_errs_reverb_batch).astype(np.float32)

    if "nc" not in _cache:
        _cache["nc"] = _install_hoist(_build())
        _cache["tabs"] = _tables()
    nc = _cache["nc"]
    bdcs, ss, ppj, pp016, i64b, ident8 = _cache["tabs"]

    # host pre-transpose: [core][it, (w4 t), (g s)] so device DMA is contiguous
    wins = (audio_batch.reshape(NCORES, WLOC, T, S)
            .reshape(NCORES, ITERS, G, 4, T, S)
            .transpose(0, 1, 3, 4, 2, 5)
            .reshape(NCORES, ITERS * 128, G * 128)
            .astype(ml_dtypes.bfloat16))
    sy = symbols_batch.reshape(NCORES, BLOC, NW)
    in_maps = []
    for c in range(NCORES):
        oh = (sy[c].T[:, :, None] == np.arange(8)).astype(np.float32).reshape(128, 64)
        in_maps.append({
            "audio": wins[c], "onehot": oh,
            "bdcs": bdcs, "ss": ss, "ppj": ppj, "pp016": pp016,
            "i64b": i64b, "ident8": ident8,
        })
    import os
    res = run_bass_kernel_spmd(nc, in_maps, core_ids=list(range(NCORES)),
                               trace=bool(os.environ.get("KTRACE")))
    _cache["last_res"] = res
    errs = np.zeros(B, np.float32)
    for c in range(NCORES):
        loss = res.results[c]["loss_out"]          # [128 w, 8 audios]
        errs[c * BLOC:(c + 1) * BLOC] = loss.sum(axis=0, dtype=np.float32)

    tot = np.float32(errs.sum())
    diff = nr_ - nn_
    inv_red = np.where(diff == 0, np.float32(1.0), diff / (nr_ - errs))
    ter = np.float32(inv_red.sum())
    denom = np.float32(B * NW)
    return (np.float32(tot / denom), tot, np.float32(ter / B),
            np.float32(nn_.sum() / denom), np.float32(nr_.sum() / denom))


# revision 15
# speedup vs baseline: 3.4660x; 1.0833x over previous
"""Trainium2 Bass kernel for nn_DecodingLoss (cepstrum decoding loss).

Math (per 4096-sample window):
  cep = irfft(log(|rfft(x)| + eps))[DELAYS]; softargmax(beta=1e10) ~= hard argmax;
  loss = clip(|idx - symbol|,0,1) = 1[argmax != symbol]; per-audio sums -> 5 scalars.

Kernel strategy (8 cores, pure data parallel over the batch dim; 1024 windows/core):
  FFT 4096 = 32 x 128 Cooley-Tukey, n = 128 t + s (t<32, s<128), k = u + 32 v.
  stage1 (PE): per 4-window group, stationary = x4 [(w4 t), s], moving = block-diag
    W32 table -> psA = A^T[s, (j, w4)] directly (no transpose step). Real-input
    hermitian symmetry: only u=0..16 kept (u0/u16 real), 32 j-cols per window.
    The PSUM->SBUF copy scatters to a j-major `at` so every stage2 moving operand
    is a contiguous 256-column slice (strided movings halve PE stream rate).
  stage2 (PE): for q=1..15 the conjugate k-sets {q+32v} and {32-q+32v} share
    moving operands rre/rim; 128-wide stationaries emit Re of both sets into one
    psX region and Im into another -> |X|^2 = aligned full-width adds. Two q's
    batched per [128,1024] psX ("super-pair") to halve ACT instruction count.
    u=0/16 singleton handled via a PE stacked-identity sum (psM).
  log|X|: ACT Square(scale 2^-6) -> bf16, DVE add, ACT Ln (values centered near 0
  so bf16 is safe), per-pair bf16 projection matmuls accumulate cep[8, 256 win].
  Loss: transpose cep to [win, tap]; batched: sel = cep[sym] via one-hot mult,
  loss = min((max - sel)*1e12, 1). Host sums per-audio errors + final scalars.
  Pipelining: stage1 quads of iteration N+1 are interleaved between the supers of
  iteration N so copies never gate the PE; projections trail their super by 2.
"""
import numpy as np
import ml_dtypes

import concourse.bass as bass
import concourse.mybir as mybir
from concourse import tile
from concourse.bass_utils import run_bass_kernel_spmd

FP32 = mybir.dt.float32
BF16 = mybir.dt.bfloat16
I32 = mybir.dt.int32

B, NW, WIN = 64, 128, 4096
NCORES = 8
BLOC = B // NCORES              # 8 audio rows per core
WLOC = BLOC * NW                # 1024 windows per core
T, S = 32, 128                  # n = 128 t + s
NV = 64                         # v-grid size per k-set
ITERS = 4
WPI = WLOC // ITERS             # 256 windows per iteration
G = WPI // 4                    # 64 groups of 4 windows
DELAYS = np.array([64, 96, 128, 160, 192, 224, 256, 288])
SQ_SCALE = 2.0 ** -6            # |X|^2 scaled by 2^-12: ln output centered near 0
LN_EPS = 2.44e-14

_cache = {}


def _hoist_waits(bir_json):
    """This walrus build rejects instructions carrying attached semaphore waits
    ("Too many sync wait commands"); raw-bass style standalone EventSemaphore
    waits compile and run. Hoist every attached wait into its own
    EventSemaphore on the same engine queue; updates stay attached."""
    import json
    d = json.loads(bir_json)
    n = 0
    for fn in d["functions"]:
        for bb in fn["blocks"]:
            out = []
            for ins in bb["instructions"]:
                si = ins.get("sync_info")
                waits = (si or {}).get("on_wait") or []
                if waits and ins.get("opcode") != "EventSemaphore" and ins.get("engine"):
                    for w in waits:
                        n += 1
                        out.append({
                            "name": f"hoistw-{n}", "opcode": "EventSemaphore",
                            "engine": ins["engine"], "ins": [], "outs": [],
                            "sync_info": {"on_wait": [w], "on_update": []},
                        })
                    si["on_wait"] = []
                out.append(ins)
            bb["instructions"] = out
    return json.dumps(d).encode()


def _install_hoist(nc):
    orig = nc.to_json_bytes
    nc.to_json_bytes = lambda: _hoist_waits(orig())
    return nc


def _tables():
    t = np.arange(T)
    # BDCS [128,128]: rows (w4,t), cols (jj,w4); jj: 0=re u0, 1=re u16,
    # 2..16=re u=1..15, 17..31=im u=1..15
    blk = np.zeros((32, 32))
    blk[:, 0] = 1.0
    blk[:, 1] = np.cos(np.pi * t)
    for u in range(1, 16):
        blk[:, u + 1] = np.cos(2 * np.pi * t * u / 32.0)
        blk[:, u + 16] = -np.sin(2 * np.pi * t * u / 32.0)
    bdcs = np.zeros((128, 128))
    for w in range(4):
        bdcs[w * 32:(w + 1) * 32, w * 32:(w + 1) * 32] = blk
    perm = np.array([w4 * 32 + jj for jj in range(32) for w4 in range(4)])
    bdcs = bdcs[:, perm]

    s = np.arange(S)[:, None]
    v = np.arange(NV)[None, :]
    # ss [128, 62*128]: q=1..15 -> blocks (q-1)*4 + {SR1,SR2,SI1,SI2}; S0=60, S16=61
    ss = np.zeros((128, 62 * 128))
    for q in range(1, 16):
        phA = 2 * np.pi * s * (q + 32 * v) / 4096.0
        phB = 2 * np.pi * s * ((32 - q) + 32 * v) / 4096.0
        o = (q - 1) * 4 * 128
        ss[:, o:o + 128] = np.hstack([np.cos(phA), np.cos(phB)])         # SR1 @ rre
        ss[:, o + 128:o + 256] = np.hstack([np.sin(phA), -np.sin(phB)])   # SR2 @ rim
        ss[:, o + 256:o + 384] = np.hstack([-np.sin(phA), -np.sin(phB)])  # SI1 @ rre
        ss[:, o + 384:o + 512] = np.hstack([np.cos(phA), -np.cos(phB)])   # SI2 @ rim
    ph0 = 2 * np.pi * s * (32 * (v + 1)) / 4096.0
    ph16 = 2 * np.pi * s * (16 + 32 * v) / 4096.0
    ss[:, 60 * 128:61 * 128] = np.hstack([np.cos(ph0), -np.sin(ph0)])
    ss[:, 61 * 128:62 * 128] = np.hstack([np.cos(ph16), -np.sin(ph16)])

    vv = np.arange(NV)

    def ppcol(k):  # [64, 8]
        wk = np.where(k == 2048, 1.0, 2.0)
        return (wk[:, None] * 0.5 *
                np.cos(2 * np.pi * k[:, None] * DELAYS[None, :] / 4096.0) / 4096.0)

    ppj = np.zeros((128, 15 * 8))
    for q in range(1, 16):
        ppj[0:64, (q - 1) * 8:q * 8] = ppcol(q + 32 * vv)
        ppj[64:128, (q - 1) * 8:q * 8] = ppcol((32 - q) + 32 * vv)
    pp016 = np.zeros((64, 16))
    pp016[:, 0:8] = ppcol(32 * (vv + 1))
    pp016[:, 8:16] = ppcol(16 + 32 * vv)

    i64b = np.zeros((128, 64))
    i64b[np.arange(128), np.arange(128) % 64] = 1.0
    ident8 = np.eye(8)
    bf = ml_dtypes.bfloat16
    return (bdcs.astype(bf), ss.astype(bf), ppj.astype(bf), pp016.astype(bf),
            i64b.astype(bf), ident8.astype(np.float32))


def _build():
    nc = bass.Bass()
    audio = nc.dram_tensor("audio", [ITERS * 128, G * 128], BF16, kind="ExternalInput")
    bdcs_d = nc.dram_tensor("bdcs", [128, 128], BF16, kind="ExternalInput")
    ss_d = nc.dram_tensor("ss", [128, 62 * 128], BF16, kind="ExternalInput")
    ppj_d = nc.dram_tensor("ppj", [128, 120], BF16, kind="ExternalInput")
    pp016_d = nc.dram_tensor("pp016", [64, 16], BF16, kind="ExternalInput")
    oh_d = nc.dram_tensor("onehot", [128, 64], FP32, kind="ExternalInput")
    i64_d = nc.dram_tensor("i64b", [128, 64], BF16, kind="ExternalInput")
    id8_d = nc.dram_tensor("ident8", [8, 8], FP32, kind="ExternalInput")
    loss_out = nc.dram_tensor("loss_out", [128, 8], FP32, kind="ExternalOutput")

    with tile.TileContext(nc) as tc:
        with (
            tc.tile_pool(name="consts", bufs=1) as consts,
            tc.tile_pool(name="xt", bufs=2) as xt_pool,
            tc.tile_pool(name="at", bufs=2) as at_pool,
            tc.tile_pool(name="sq", bufs=2) as sq_pool,
            tc.tile_pool(name="m2", bufs=2) as m2_pool,
            tc.tile_pool(name="lg", bufs=4) as lg_pool,
            tc.tile_pool(name="lg0", bufs=2) as lg0_pool,
            tc.tile_pool(name="fin", bufs=2) as fin_pool,
            tc.tile_pool(name="psA", bufs=2, space="PSUM") as psA_pool,
            tc.tile_pool(name="psX", bufs=2, space="PSUM") as psX_pool,
            tc.tile_pool(name="cep", bufs=1, space="PSUM") as cep_pool,
        ):
            bdcs = consts.tile([128, 128], BF16, tag="bdcs")
            nc.sync.dma_start(bdcs[:], bdcs_d[:])
            ss = consts.tile([128, 62 * 128], BF16, tag="ss")
            nc.sync.dma_start(ss[:], ss_d[:])
            ppj = consts.tile([128, 120], BF16, tag="ppj")
            nc.sync.dma_start(ppj[:], ppj_d[:])
            pp016 = consts.tile([64, 16], BF16, tag="pp016")
            nc.sync.dma_start(pp016[:], pp016_d[:])
            onehot = consts.tile([128, 64], FP32, tag="onehot")
            nc.sync.dma_start(onehot[:], oh_d[:])
            i64b = consts.tile([128, 64], BF16, tag="i64b")
            nc.sync.dma_start(i64b[:], i64_d[:])
            ident8 = consts.tile([8, 8], FP32, tag="ident8")
            nc.sync.dma_start(ident8[:], id8_d[:])
            epsb = consts.tile([128, 1], FP32, tag="epsb")
            nc.vector.memset(epsb[:], LN_EPS)
            cepT = consts.tile([128, 64], FP32, tag="cepT")

            def sblk(b):  # stationary block b of ss
                return ss[:, b * 128:(b + 1) * 128]

            xts = [None, None]
            ats = [None, None]

            def dma_xt(it):
                xt = xt_pool.tile([128, G * 128], BF16, tag="xt")
                for c in range(4):
                    nc.sync.dma_start(
                        xt[:, c * 2048:(c + 1) * 2048],
                        audio[it * 128:(it + 1) * 128, c * 2048:(c + 1) * 2048])
                xts[it % 2] = xt
                ats[it % 2] = at_pool.tile([128, G * 128], BF16, tag="at", name="at")

            def quad(it, qd):
                # stage1: 4 groups -> psA [s, (g, j, w4)], scatter-copy j-major
                xt, at = xts[it % 2], ats[it % 2]
                g0 = qd * 4
                psA = psA_pool.tile([128, 512], FP32, tag="psA")
                for g in range(g0, g0 + 4):
                    nc.tensor.matmul(psA[:, (g - g0) * 128:(g - g0 + 1) * 128],
                                     xt[:, g * 128:(g + 1) * 128], bdcs[:],
                                     start=True, stop=True)
                atv_j = at[:].rearrange("s (j g w4) -> s j g w4", j=32, w4=4)
                dst = atv_j[:, :, g0:g0 + 4, :]
                src = psA[:].rearrange("s (g j w4) -> s j g w4", g=4, w4=4)
                if qd % 2 == 0:
                    nc.vector.tensor_copy(dst, src)
                else:
                    nc.scalar.activation(dst, src,
                                         mybir.ActivationFunctionType.Copy)

            # ---- prologue: iteration 0 stage1 ----
            dma_xt(0)
            for qd in range(16):
                quad(0, qd)

            for it in range(ITERS):
                at = ats[it % 2]
                if it + 1 < ITERS:
                    dma_xt(it + 1)

                cep = cep_pool.tile([128, 512], FP32, tag="cep")
                projq = []
                nproj = 17
                emitted = [0]

                def emit_proj():
                    projq.pop(0)()
                    emitted[0] += 1

                def mk_proj(stat, lgt, cep=cep):
                    def fn():
                        nc.tensor.matmul(cep[0:8, 0:256], stat, lgt,
                                         start=(emitted[0] == 0),
                                         stop=(emitted[0] == nproj - 1))
                    return fn

                sq7 = [None]

                def super_pair(s):
                    # two pairs per psX bank-pair; s=7 packs (q=15, u0/u16)
                    psX = psX_pool.tile([128, 1024], FP32, tag="psX")
                    if s < 7:
                        qa, qb = 2 * s + 1, 2 * s + 2
                        for h, q in ((0, qa), (1, qb)):
                            rre = at[:, (q + 1) * 256:(q + 2) * 256]
                            o = (q - 1) * 4
                            nc.tensor.matmul(psX[:, h * 512:h * 512 + 256],
                                             sblk(o), rre, start=True, stop=False)
                            nc.tensor.matmul(psX[:, h * 512 + 256:h * 512 + 512],
                                             sblk(o + 2), rre, start=True, stop=False)
                        for h, q in ((0, qa), (1, qb)):
                            rim = at[:, (q + 16) * 256:(q + 17) * 256]
                            o = (q - 1) * 4
                            nc.tensor.matmul(psX[:, h * 512:h * 512 + 256],
                                             sblk(o + 1), rim, start=False, stop=True)
                            nc.tensor.matmul(psX[:, h * 512 + 256:h * 512 + 512],
                                             sblk(o + 3), rim, start=False, stop=True)
                    else:
                        q = 15
                        rre = at[:, (q + 1) * 256:(q + 2) * 256]
                        rim = at[:, (q + 16) * 256:(q + 17) * 256]
                        o = (q - 1) * 4
                        nc.tensor.matmul(psX[:, 0:256], sblk(o), rre, start=True, stop=False)
                        nc.tensor.matmul(psX[:, 256:512], sblk(o + 2), rre, start=True, stop=False)
                        nc.tensor.matmul(psX[:, 512:768], sblk(60), at[:, 0:256],
                                         start=True, stop=True)
                        nc.tensor.matmul(psX[:, 768:1024], sblk(61), at[:, 256:512],
                                         start=True, stop=True)
                        nc.tensor.matmul(psX[:, 0:256], sblk(o + 1), rim, start=False, stop=True)
                        nc.tensor.matmul(psX[:, 256:512], sblk(o + 3), rim, start=False, stop=True)
                    sq = sq_pool.tile([128, 1024], BF16, tag="sq")
                    nc.scalar.activation(sq[:], psX[:],
                                         mybir.ActivationFunctionType.Square,
                                         scale=SQ_SCALE)
                    sqv = sq[:].rearrange("s (p2 h c) -> s p2 h c", p2=2, h=2)
                    m2 = m2_pool.tile([128, 512], BF16, tag="m2")
                    m2v = m2[:].rearrange("s (p2 c) -> s p2 c", p2=2)
                    if s < 7:
                        nc.vector.tensor_add(m2v, sqv[:, :, 0, :], sqv[:, :, 1, :])
                        lg = lg_pool.tile([128, 512], BF16, tag="lg")
                        nc.scalar.activation(lg[:], m2[:],
                                             mybir.ActivationFunctionType.Ln,
                                             bias=epsb[:])
                        projq.append(mk_proj(ppj[:, (qa - 1) * 8:qa * 8], lg[:, 0:256]))
                        projq.append(mk_proj(ppj[:, (qb - 1) * 8:qb * 8], lg[:, 256:512]))
                    else:
                        nc.vector.tensor_add(m2[:, 0:256], sq[:, 0:256], sq[:, 256:512])
                        lg = lg_pool.tile([128, 512], BF16, tag="lg")
                        nc.scalar.activation(lg[:, 0:256], m2[:, 0:256],
                                             mybir.ActivationFunctionType.Ln,
                                             bias=epsb[:])
                        projq.append(mk_proj(ppj[:, 14 * 8:15 * 8], lg[:, 0:256]))
                        sq7[0] = sq

                def mk_q0_tail(cep=cep):
                    def fn():
                        sq = sq7[0]
                        psM = psX_pool.tile([128, 1024], FP32, tag="psX")
                        nc.tensor.matmul(psM[0:64, 0:512], i64b[:], sq[:, 512:1024],
                                         start=True, stop=True)
                        lg0 = lg0_pool.tile([64, 512], BF16, tag="lg0")
                        nc.scalar.activation(lg0[:], psM[0:64, 0:512],
                                             mybir.ActivationFunctionType.Ln,
                                             bias=epsb[0:64])
                        mk_proj(pp016[:, 0:8], lg0[:, 0:256], cep)()
                        emitted[0] += 1
                        mk_proj(pp016[:, 8:16], lg0[:, 256:512], cep)()
                        emitted[0] += 1
                    return fn

                def mk_fin(it, cep=cep):
                    def fn():
                        cep_sb = fin_pool.tile([8, 256], FP32, tag="cep_sb")
                        nc.scalar.activation(cep_sb[:], cep[0:8, 0:256],
                                             mybir.ActivationFunctionType.Copy)
                        for c in range(2):
                            gc = it * 2 + c
                            psC = cep[:, 256 + c * 8:256 + (c + 1) * 8]
                            nc.tensor.transpose(psC, cep_sb[:, c * 128:(c + 1) * 128],
                                                ident8[:])
                            nc.vector.tensor_copy(cepT[:, gc * 8:(gc + 1) * 8], psC)
                    return fn

                for s in range(8):
                    super_pair(s)
                    if it + 1 < ITERS:
                        quad(it + 1, 2 * s)
                        quad(it + 1, 2 * s + 1)
                    if s >= 2:
                        emit_proj()
                        emit_proj()

                # tail: supers 6,7 projections, q0 chain, cep finalize.
                # Emitted inline; the next iteration's first supers only depend
                # on at(it+1), so the PE keeps streaming after short stalls.
                while projq:
                    emit_proj()
                mk_q0_tail()()
                mk_fin(it)()

            # batched loss over all 1024 windows: [128 w, 8 audios]
            tmp = fin_pool.tile([128, 64], FP32, tag="tmp")
            nc.vector.tensor_mul(tmp[:], cepT[:], onehot[:])
            sel = fin_pool.tile([128, 8], FP32, tag="sel")
            nc.vector.reduce_sum(sel[:], tmp[:].rearrange("p (a j) -> p a j", j=8),
                                 axis=mybir.AxisListType.X)
            mx = fin_pool.tile([128, 8], FP32, tag="mx")
            nc.vector.reduce_max(mx[:], cepT[:].rearrange("p (a j) -> p a j", j=8),
                                 axis=mybir.AxisListType.X)
            df = fin_pool.tile([128, 8], FP32, tag="df")
            nc.vector.tensor_sub(df[:], mx[:], sel[:])
            df2 = fin_pool.tile([128, 8], FP32, tag="df2")
            nc.vector.tensor_scalar_mul(df2[:], df[:], 1e12)
            ls = fin_pool.tile([128, 8], FP32, tag="ls")
            nc.vector.tensor_scalar_min(ls[:], df2[:], 1.0)
            nc.sync.dma_start(loss_out[:], ls[:])
    return nc


def kernel(audio_batch, symbols_batch, num_errs_no_reverb_batch,
           num_errs_reverb_batch):
    audio_batch = np.asarray(audio_batch)
    symbols_batch = np.asarray(symbols_batch, dtype=np.int32)
    nn_ = np.asarray(num_errs_no_reverb_batch).astype(np.float32)
    nr_ = np.asarray(num_errs_reverb_batch).astype(np.float32)

    if "nc" not in _cache:
        _cache["nc"] = _install_hoist(_build())
        _cache["tabs"] = _tables()
    nc = _cache["nc"]
    bdcs, ss, ppj, pp016, i64b, ident8 = _cache["tabs"]

    # host pre-transpose: [core][it, (w4 t), (g s)] so device DMA is contiguous
    wins = (audio_batch.reshape(NCORES, WLOC, T, S)
            .reshape(NCORES, ITERS, G, 4, T, S)
            .transpose(0, 1, 3, 4, 2, 5)
            .reshape(NCORES, ITERS * 128, G * 128)
            .astype(ml_dtypes.bfloat16))
    sy = symbols_batch.reshape(NCORES, BLOC, NW)
    in_maps = []
    for c in range(NCORES):
        oh = (sy[c].T[:, :, None] == np.arange(8)).astype(np.float32).reshape(128, 64)
        in_maps.append({
            "audio": wins[c], "onehot": oh,
            "bdcs": bdcs, "ss": ss, "ppj": ppj, "pp016": pp016,
            "i64b": i64b, "ident8": ident8,
        })
    import os
    res = run_bass_kernel_spmd(nc, in_maps, core_ids=list(range(NCORES)),
                               trace=bool(os.environ.get("KTRACE")))
    _cache["last_res"] = res
    errs = np.zeros(B, np.float32)
    for c in range(NCORES):
        loss = res.results[c]["loss_out"]          # [128 w, 8 audios]
        errs[c * BLOC:(c + 1) * BLOC] = loss.sum(axis=0, dtype=np.float32)

    tot = np.float32(errs.sum())
    diff = nr_ - nn_
    inv_red = np.where(diff == 0, np.float32(1.0), diff / (nr_ - errs))
    ter = np.float32(inv_red.sum())
    denom = np.float32(B * NW)
    return (np.float32(tot / denom), tot, np.float32(ter / B),
            np.float32(nn_.sum() / denom), np.float32(nr_.sum() / denom))


# revision 23
# speedup vs baseline: 3.5519x; 1.0248x over previous
"""Trainium2 Bass kernel for nn_DecodingLoss (cepstrum decoding loss).

Math (per 4096-sample window):
  cep = irfft(log(|rfft(x)| + eps))[DELAYS]; softargmax(beta=1e10) ~= hard argmax;
  loss = clip(|idx - symbol|,0,1) = 1[argmax != symbol]; per-audio sums -> 5 scalars.

Kernel strategy (8 cores, pure data parallel over the batch dim; 1024 windows/core):
  FFT 4096 = 32 x 128 Cooley-Tukey, n = 128 t + s (t<32, s<128), k = u + 32 v.
  stage1 (PE): per 4-window group, stationary = x4 [(w4 t), s], moving = block-diag
    W32 table -> psA = A^T[s, (j, w4)] directly (no transpose step). Real-input
    hermitian symmetry: only u=0..16 kept (u0/u16 real), 32 j-cols per window.
    The PSUM->SBUF copy scatters to a j-major `at` so every stage2 moving operand
    is a contiguous 256-column slice (strided movings halve PE stream rate).
  stage2 (PE): for q=1..15 the conjugate k-sets {q+32v} and {32-q+32v} share
    moving operands rre/rim; 128-wide stationaries emit Re of both sets into one
    psX region and Im into another -> |X|^2 = aligned full-width adds. Two q's
    batched per [128,1024] psX ("super-pair") to halve ACT instruction count.
    u=0/16 singleton handled via a PE stacked-identity sum (psM).
  log|X|: ACT Square(scale 2^-6) -> bf16, DVE add, ACT Ln (values centered near 0
  so bf16 is safe), per-pair bf16 projection matmuls accumulate cep[8, 256 win].
  Loss: transpose cep to [win, tap]; batched: sel = cep[sym] via one-hot mult,
  loss = min((max - sel)*1e12, 1). Host sums per-audio errors + final scalars.
  Pipelining: stage1 quads of iteration N+1 are interleaved between the supers of
  iteration N so copies never gate the PE; projections trail their super by 2.
"""
import numpy as np
import ml_dtypes

import concourse.bass as bass
import concourse.mybir as mybir
from concourse import tile
from concourse.bass_utils import run_bass_kernel_spmd

FP32 = mybir.dt.float32
BF16 = mybir.dt.bfloat16
I32 = mybir.dt.int32

B, NW, WIN = 64, 128, 4096
NCORES = 8
BLOC = B // NCORES              # 8 audio rows per core
WLOC = BLOC * NW                # 1024 windows per core
T, S = 32, 128                  # n = 128 t + s
NV = 64                         # v-grid size per k-set
ITERS = 4
WPI = WLOC // ITERS             # 256 windows per iteration
G = WPI // 4                    # 64 groups of 4 windows
DELAYS = np.array([64, 96, 128, 160, 192, 224, 256, 288])
SQ_SCALE = 2.0 ** -6            # |X|^2 scaled by 2^-12: ln output centered near 0
LN_EPS = 2.44e-14

_cache = {}


def _hoist_waits(bir_json):
    """This walrus build rejects instructions carrying attached semaphore waits
    ("Too many sync wait commands"); raw-bass style standalone EventSemaphore
    waits compile and run. Hoist every attached wait into its own
    EventSemaphore on the same engine queue; updates stay attached."""
    import json
    d = json.loads(bir_json)
    n = 0
    for fn in d["functions"]:
        for bb in fn["blocks"]:
            out = []
            for ins in bb["instructions"]:
                si = ins.get("sync_info")
                waits = (si or {}).get("on_wait") or []
                if waits and ins.get("opcode") != "EventSemaphore" and ins.get("engine"):
                    for w in waits:
                        n += 1
                        out.append({
                            "name": f"hoistw-{n}", "opcode": "EventSemaphore",
                            "engine": ins["engine"], "ins": [], "outs": [],
                            "sync_info": {"on_wait": [w], "on_update": []},
                        })
                    si["on_wait"] = []
                out.append(ins)
            bb["instructions"] = out
    return json.dumps(d).encode()


def _install_hoist(nc):
    orig = nc.to_json_bytes
    nc.to_json_bytes = lambda: _hoist_waits(orig())
    return nc


def _tables():
    t = np.arange(T)
    # BDCS [128,128]: rows (w4,t), cols (jj,w4); jj: 0=re u0, 1=re u16,
    # 2..16=re u=1..15, 17..31=im u=1..15
    blk = np.zeros((32, 32))
    blk[:, 0] = 1.0
    blk[:, 1] = np.cos(np.pi * t)
    for u in range(1, 16):
        blk[:, u + 1] = np.cos(2 * np.pi * t * u / 32.0)
        blk[:, u + 16] = -np.sin(2 * np.pi * t * u / 32.0)
    bdcs = np.zeros((128, 128))
    for w in range(4):
        bdcs[w * 32:(w + 1) * 32, w * 32:(w + 1) * 32] = blk
    perm = np.array([w4 * 32 + jj for jj in range(32) for w4 in range(4)])
    bdcs = bdcs[:, perm]

    s = np.arange(S)[:, None]
    v = np.arange(NV)[None, :]
    # ss [128, 62*128]: q=1..15 -> blocks (q-1)*4 + {SR1,SR2,SI1,SI2}; S0=60, S16=61
    ss = np.zeros((128, 62 * 128))
    for q in range(1, 16):
        phA = 2 * np.pi * s * (q + 32 * v) / 4096.0
        phB = 2 * np.pi * s * ((32 - q) + 32 * v) / 4096.0
        o = (q - 1) * 4 * 128
        ss[:, o:o + 128] = np.hstack([np.cos(phA), np.cos(phB)])         # SR1 @ rre
        ss[:, o + 128:o + 256] = np.hstack([np.sin(phA), -np.sin(phB)])   # SR2 @ rim
        ss[:, o + 256:o + 384] = np.hstack([-np.sin(phA), -np.sin(phB)])  # SI1 @ rre
        ss[:, o + 384:o + 512] = np.hstack([np.cos(phA), -np.cos(phB)])   # SI2 @ rim
    ph0 = 2 * np.pi * s * (32 * (v + 1)) / 4096.0
    ph16 = 2 * np.pi * s * (16 + 32 * v) / 4096.0
    ss[:, 60 * 128:61 * 128] = np.hstack([np.cos(ph0), -np.sin(ph0)])
    ss[:, 61 * 128:62 * 128] = np.hstack([np.cos(ph16), -np.sin(ph16)])

    vv = np.arange(NV)

    def ppcol(k):  # [64, 8]
        wk = np.where(k == 2048, 1.0, 2.0)
        return (wk[:, None] * 0.5 *
                np.cos(2 * np.pi * k[:, None] * DELAYS[None, :] / 4096.0) / 4096.0)

    ppj = np.zeros((128, 15 * 8))
    for q in range(1, 16):
        ppj[0:64, (q - 1) * 8:q * 8] = ppcol(q + 32 * vv)
        ppj[64:128, (q - 1) * 8:q * 8] = ppcol((32 - q) + 32 * vv)
    pp016 = np.zeros((64, 16))
    pp016[:, 0:8] = ppcol(32 * (vv + 1))
    pp016[:, 8:16] = ppcol(16 + 32 * vv)

    i64b = np.zeros((128, 64))
    i64b[np.arange(128), np.arange(128) % 64] = 1.0
    ident8 = np.eye(8)
    bf = ml_dtypes.bfloat16
    return (bdcs.astype(bf), ss.astype(bf), ppj.astype(bf), pp016.astype(bf),
            i64b.astype(bf), ident8.astype(np.float32))


def _build():
    nc = bass.Bass()
    audio = nc.dram_tensor("audio", [ITERS * 128, G * 128], BF16, kind="ExternalInput")
    bdcs_d = nc.dram_tensor("bdcs", [128, 128], BF16, kind="ExternalInput")
    ss_d = nc.dram_tensor("ss", [128, 62 * 128], BF16, kind="ExternalInput")
    ppj_d = nc.dram_tensor("ppj", [128, 120], BF16, kind="ExternalInput")
    pp016_d = nc.dram_tensor("pp016", [64, 16], BF16, kind="ExternalInput")
    oh_d = nc.dram_tensor("onehot", [128, 64], FP32, kind="ExternalInput")
    i64_d = nc.dram_tensor("i64b", [128, 64], BF16, kind="ExternalInput")
    id8_d = nc.dram_tensor("ident8", [8, 8], FP32, kind="ExternalInput")
    loss_out = nc.dram_tensor("loss_out", [128, 8], FP32, kind="ExternalOutput")
    cep_dbg = nc.dram_tensor("cep_dbg", [128, 64], FP32, kind="ExternalOutput")
    at_dbg = nc.dram_tensor("at_dbg", [128, G * 128], BF16, kind="ExternalOutput")
    lg_dbg = nc.dram_tensor("lg_dbg", [128, 512], BF16, kind="ExternalOutput")

    with tile.TileContext(nc) as tc:
        with (
            tc.tile_pool(name="consts", bufs=1) as consts,
            tc.tile_pool(name="xt", bufs=2) as xt_pool,
            tc.tile_pool(name="at", bufs=2) as at_pool,
            tc.tile_pool(name="sq", bufs=2) as sq_pool,
            tc.tile_pool(name="m2", bufs=2) as m2_pool,
            tc.tile_pool(name="lg", bufs=4) as lg_pool,
            tc.tile_pool(name="lg0", bufs=2) as lg0_pool,
            tc.tile_pool(name="fin", bufs=2) as fin_pool,
            tc.tile_pool(name="psA", bufs=2, space="PSUM") as psA_pool,
            tc.tile_pool(name="psX", bufs=2, space="PSUM") as psX_pool,
            tc.tile_pool(name="cep", bufs=1, space="PSUM") as cep_pool,
        ):
            bdcs = consts.tile([128, 128], BF16, tag="bdcs")
            nc.sync.dma_start(bdcs[:], bdcs_d[:])
            ss = consts.tile([128, 62 * 128], BF16, tag="ss")
            nc.sync.dma_start(ss[:], ss_d[:])
            ppj = consts.tile([128, 120], BF16, tag="ppj")
            nc.sync.dma_start(ppj[:], ppj_d[:])
            pp016 = consts.tile([64, 16], BF16, tag="pp016")
            nc.sync.dma_start(pp016[:], pp016_d[:])
            onehot = consts.tile([128, 64], FP32, tag="onehot")
            nc.sync.dma_start(onehot[:], oh_d[:])
            i64b = consts.tile([128, 64], BF16, tag="i64b")
            nc.sync.dma_start(i64b[:], i64_d[:])
            ident8 = consts.tile([8, 8], FP32, tag="ident8")
            nc.sync.dma_start(ident8[:], id8_d[:])
            epsb = consts.tile([128, 1], FP32, tag="epsb")
            nc.vector.memset(epsb[:], LN_EPS)
            cepT = consts.tile([128, 64], FP32, tag="cepT")

            def sblk(b):  # stationary block b of ss
                return ss[:, b * 128:(b + 1) * 128]

            xts = [None, None]
            ats = [None, None]

            def dma_xt(it):
                xt = xt_pool.tile([128, G * 128], BF16, tag="xt")
                for c in range(4):
                    nc.sync.dma_start(
                        xt[:, c * 2048:(c + 1) * 2048],
                        audio[it * 128:(it + 1) * 128, c * 2048:(c + 1) * 2048])
                xts[it % 2] = xt
                ats[it % 2] = at_pool.tile([128, G * 128], BF16, tag="at", name="at")

            def quad(it, qd):
                # stage1: 4 groups -> psA [s, (g, j, w4)], scatter-copy j-major
                xt, at = xts[it % 2], ats[it % 2]
                g0 = qd * 4
                psA = psA_pool.tile([128, 512], FP32, tag="psA")
                for g in range(g0, g0 + 4):
                    nc.tensor.matmul(psA[:, (g - g0) * 128:(g - g0 + 1) * 128],
                                     xt[:, g * 128:(g + 1) * 128], bdcs[:],
                                     start=True, stop=True)
                atv_j = at[:].rearrange("s (j g w4) -> s j g w4", j=32, w4=4)
                dst = atv_j[:, :, g0:g0 + 4, :]
                src = psA[:].rearrange("s (g j w4) -> s j g w4", g=4, w4=4)
                if qd % 2 == 0:
                    nc.vector.tensor_copy(dst, src)
                else:
                    nc.scalar.activation(dst, src,
                                         mybir.ActivationFunctionType.Copy)

            # ---- prologue: iteration 0 stage1 ----
            dma_xt(0)
            for qd in range(16):
                quad(0, qd)

            for it in range(ITERS):
                at = ats[it % 2]
                if it + 1 < ITERS:
                    dma_xt(it + 1)

                cep = cep_pool.tile([128, 512], FP32, tag="cep")
                projq = []
                nproj = 17
                emitted = [0]

                def emit_proj():
                    projq.pop(0)()
                    emitted[0] += 1

                def mk_proj(stat, lgt, cep=cep):
                    def fn():
                        nc.tensor.matmul(cep[0:8, 0:256], stat, lgt,
                                         start=(emitted[0] == 0),
                                         stop=(emitted[0] == nproj - 1))
                    return fn

                sq7 = [None]

                def super_pair(s):
                    # two pairs per psX bank-pair; s=7 packs (q=15, u0/u16)
                    psX = psX_pool.tile([128, 1024], FP32, tag="psX")
                    if s < 7:
                        qa, qb = 2 * s + 1, 2 * s + 2
                        for h, q in ((0, qa), (1, qb)):
                            rre = at[:, (q + 1) * 256:(q + 2) * 256]
                            rim = at[:, (q + 16) * 256:(q + 17) * 256]
                            o = (q - 1) * 4
                            nc.tensor.matmul(psX[:, h * 512:h * 512 + 256],
                                             sblk(o), rre, start=True, stop=False)
                            nc.tensor.matmul(psX[:, h * 512:h * 512 + 256],
                                             sblk(o + 1), rim, start=False, stop=True)
                            nc.tensor.matmul(psX[:, h * 512 + 256:h * 512 + 512],
                                             sblk(o + 2), rre, start=True, stop=False)
                            nc.tensor.matmul(psX[:, h * 512 + 256:h * 512 + 512],
                                             sblk(o + 3), rim, start=False, stop=True)
                    else:
                        q = 15
                        rre = at[:, (q + 1) * 256:(q + 2) * 256]
                        rim = at[:, (q + 16) * 256:(q + 17) * 256]
                        o = (q - 1) * 4
                        nc.tensor.matmul(psX[:, 0:256], sblk(o), rre, start=True, stop=False)
                        nc.tensor.matmul(psX[:, 0:256], sblk(o + 1), rim, start=False, stop=True)
                        nc.tensor.matmul(psX[:, 256:512], sblk(o + 2), rre, start=True, stop=False)
                        nc.tensor.matmul(psX[:, 256:512], sblk(o + 3), rim, start=False, stop=True)
                        nc.tensor.matmul(psX[:, 512:768], sblk(60), at[:, 0:256],
                                         start=True, stop=True)
                        nc.tensor.matmul(psX[:, 768:1024], sblk(61), at[:, 256:512],
                                         start=True, stop=True)
                    sq = sq_pool.tile([128, 1024], BF16, tag="sq")
                    nc.scalar.activation(sq[:], psX[:],
                                         mybir.ActivationFunctionType.Square,
                                         scale=SQ_SCALE)
                    m2 = m2_pool.tile([128, 512], BF16, tag="m2")
                    if s < 7:
                        nc.vector.tensor_add(m2[:, 0:256], sq[:, 0:256], sq[:, 256:512])
                        nc.vector.tensor_add(m2[:, 256:512], sq[:, 512:768], sq[:, 768:1024])
                        lg = lg_pool.tile([128, 512], BF16, tag="lg")
                        nc.scalar.activation(lg[:], m2[:],
                                             mybir.ActivationFunctionType.Ln,
                                             bias=epsb[:])
                        projq.append(mk_proj(ppj[:, (qa - 1) * 8:qa * 8], lg[:, 0:256]))
                        projq.append(mk_proj(ppj[:, (qb - 1) * 8:qb * 8], lg[:, 256:512]))
                        if it == ITERS - 1 and s == 0:
                            nc.sync.dma_start(lg_dbg[:], lg[:])
                    else:
                        nc.vector.tensor_add(m2[:, 0:256], sq[:, 0:256], sq[:, 256:512])
                        lg = lg_pool.tile([128, 512], BF16, tag="lg")
                        nc.scalar.activation(lg[:, 0:256], m2[:, 0:256],
                                             mybir.ActivationFunctionType.Ln,
                                             bias=epsb[:])
                        projq.append(mk_proj(ppj[:, 14 * 8:15 * 8], lg[:, 0:256]))
                        sq7[0] = sq

                def mk_q0_tail(cep=cep):
                    def fn():
                        sq = sq7[0]
                        psM = psX_pool.tile([128, 1024], FP32, tag="psX")
                        nc.tensor.matmul(psM[0:64, 0:512], i64b[:], sq[:, 512:1024],
                                         start=True, stop=True)
                        lg0 = lg0_pool.tile([64, 512], BF16, tag="lg0")
                        nc.scalar.activation(lg0[:], psM[0:64, 0:512],
                                             mybir.ActivationFunctionType.Ln,
                                             bias=epsb[0:64])
                        mk_proj(pp016[:, 0:8], lg0[:, 0:256], cep)()
                        emitted[0] += 1
                        mk_proj(pp016[:, 8:16], lg0[:, 256:512], cep)()
                        emitted[0] += 1
                    return fn

                def mk_fin(it, cep=cep):
                    def fn():
                        cep_sb = fin_pool.tile([8, 256], FP32, tag="cep_sb")
                        nc.scalar.activation(cep_sb[:], cep[0:8, 0:256],
                                             mybir.ActivationFunctionType.Copy)
                        for c in range(2):
                            gc = it * 2 + c
                            psC = cep[:, 256 + c * 8:256 + (c + 1) * 8]
                            nc.tensor.transpose(psC, cep_sb[:, c * 128:(c + 1) * 128],
                                                ident8[:])
                            nc.vector.tensor_copy(cepT[:, gc * 8:(gc + 1) * 8], psC)
                    return fn

                for s in range(8):
                    super_pair(s)
                    if it + 1 < ITERS:
                        quad(it + 1, 2 * s)
                        quad(it + 1, 2 * s + 1)
                    if s >= 2:
                        emit_proj()
                        emit_proj()

                # tail: supers 6,7 projections, q0 chain, cep finalize.
                # Emitted inline; the next iteration's first supers only depend
                # on at(it+1), so the PE keeps streaming after short stalls.
                while projq:
                    emit_proj()
                mk_q0_tail()()
                mk_fin(it)()

            # batched loss over all 1024 windows: [128 w, 8 audios]
            tmp = fin_pool.tile([128, 64], FP32, tag="tmp")
            nc.vector.tensor_mul(tmp[:], cepT[:], onehot[:])
            sel = fin_pool.tile([128, 8], FP32, tag="sel")
            nc.vector.reduce_sum(sel[:], tmp[:].rearrange("p (a j) -> p a j", j=8),
                                 axis=mybir.AxisListType.X)
            mx = fin_pool.tile([128, 8], FP32, tag="mx")
            nc.vector.reduce_max(mx[:], cepT[:].rearrange("p (a j) -> p a j", j=8),
                                 axis=mybir.AxisListType.X)
            df = fin_pool.tile([128, 8], FP32, tag="df")
            nc.vector.tensor_sub(df[:], mx[:], sel[:])
            df2 = fin_pool.tile([128, 8], FP32, tag="df2")
            nc.vector.tensor_scalar_mul(df2[:], df[:], 1e12)
            ls = fin_pool.tile([128, 8], FP32, tag="ls")
            nc.vector.tensor_scalar_min(ls[:], df2[:], 1.0)
            nc.sync.dma_start(loss_out[:], ls[:])
            nc.sync.dma_start(cep_dbg[:], cepT[:])
            nc.sync.dma_start(at_dbg[:], ats[(ITERS - 1) % 2][:])
    return nc


def kernel(audio_batch, symbols_batch, num_errs_no_reverb_batch,
           num_errs_reverb_batch):
    audio_batch = np.asarray(audio_batch)
    symbols_batch = np.asarray(symbols_batch, dtype=np.int32)
    nn_ = np.asarray(num_errs_no_reverb_batch).astype(np.float32)
    nr_ = np.asarray(num_errs_reverb_batch).astype(np.float32)

    if "nc" not in _cache:
        _cache["nc"] = _install_hoist(_build())
        _cache["tabs"] = _tables()
    nc = _cache["nc"]
    bdcs, ss, ppj, pp016, i64b, ident8 = _cache["tabs"]

    # host pre-transpose: [core][it, (w4 t), (g s)] so device DMA is contiguous
    wins = (audio_batch.reshape(NCORES, WLOC, T, S)
            .reshape(NCORES, ITERS, G, 4, T, S)
            .transpose(0, 1, 3, 4, 2, 5)
            .reshape(NCORES, ITERS * 128, G * 128)
            .astype(ml_dtypes.bfloat16))
    sy = symbols_batch.reshape(NCORES, BLOC, NW)
    in_maps = []
    for c in range(NCORES):
        oh = (sy[c].T[:, :, None] == np.arange(8)).astype(np.float32).reshape(128, 64)
        in_maps.append({
            "audio": wins[c], "onehot": oh,
            "bdcs": bdcs, "ss": ss, "ppj": ppj, "pp016": pp016,
            "i64b": i64b, "ident8": ident8,
        })
    import os
    res = run_bass_kernel_spmd(nc, in_maps, core_ids=list(range(NCORES)),
                               trace=bool(os.environ.get("KTRACE")))
    _cache["last_res"] = res
    errs = np.zeros(B, np.float32)
    for c in range(NCORES):
        loss = res.results[c]["loss_out"]          # [128 w, 8 audios]
        errs[c * BLOC:(c + 1) * BLOC] = loss.sum(axis=0, dtype=np.float32)

    tot = np.float32(errs.sum())
    diff = nr_ - nn_
    inv_red = np.where(diff == 0, np.float32(1.0), diff / (nr_ - errs))
    ter = np.float32(inv_red.sum())
    denom = np.float32(B * NW)
    return (np.float32(tot / denom), tot, np.float32(ter / B),
            np.float32(nn_.sum() / denom), np.float32(nr_.sum() / denom))


# revision 28
# speedup vs baseline: 4.0217x; 1.1323x over previous
"""Trainium2 Bass kernel for nn_DecodingLoss (cepstrum decoding loss).

Math (per 4096-sample window):
  cep = irfft(log(|rfft(x)| + eps))[DELAYS]; softargmax(beta=1e10) ~= hard argmax;
  loss = clip(|idx - symbol|,0,1) = 1[argmax != symbol]; per-audio sums -> 5 scalars.

Kernel strategy (8 cores, pure data parallel over the batch dim; 1024 windows/core):
  FFT 4096 = 32 x 128 Cooley-Tukey, n = 128 t + s (t<32, s<128), k = u + 32 v.
  stage1 (PE): per 4-window group, stationary = x4 [(w4 t), s], moving = block-diag
    W32 table -> psA = A^T[s, (j, w4)] directly (no transpose step). Real-input
    hermitian symmetry: only u=0..16 kept (u0/u16 real), 32 j-cols per window.
    The PSUM->SBUF copy scatters to a j-major `at` so every stage2 moving operand
    is a contiguous 256-column slice (strided movings halve PE stream rate).
  stage2 (PE): for q=1..15 the conjugate k-sets {q+32v} and {32-q+32v} share
    moving operands rre/rim; 128-wide stationaries emit Re of both sets into one
    psX region and Im into another -> |X|^2 = aligned full-width adds. Two q's
    batched per [128,1024] psX ("super-pair") to halve ACT instruction count.
    u=0/16 singleton handled via a PE stacked-identity sum (psM).
  log|X|: ACT Square(scale 2^-6) -> bf16, DVE add, ACT Ln (values centered near 0
  so bf16 is safe), per-pair bf16 projection matmuls accumulate cep[8, 256 win].
  Loss: transpose cep to [win, tap]; batched: sel = cep[sym] via one-hot mult,
  loss = min((max - sel)*1e12, 1). Host sums per-audio errors + final scalars.
  Pipelining: stage1 quads of iteration N+1 are interleaved between the supers of
  iteration N so copies never gate the PE; projections trail their super by 2.
"""
import numpy as np
import ml_dtypes

import concourse.bass as bass
import concourse.mybir as mybir
from concourse import tile
from concourse.bass_utils import run_bass_kernel_spmd

FP32 = mybir.dt.float32
BF16 = mybir.dt.bfloat16
I32 = mybir.dt.int32

B, NW, WIN = 64, 128, 4096
NCORES = 8
BLOC = B // NCORES              # 8 audio rows per core
WLOC = BLOC * NW                # 1024 windows per core
T, S = 32, 128                  # n = 128 t + s
NV = 64                         # v-grid size per k-set
ITERS = 4
WPI = WLOC // ITERS             # 256 windows per iteration
G = WPI // 4                    # 64 groups of 4 windows
DELAYS = np.array([64, 96, 128, 160, 192, 224, 256, 288])
SQ_SCALE = 2.0 ** -6            # |X|^2 scaled by 2^-12: ln output centered near 0
LN_EPS = 2.44e-14

_cache = {}


def _hoist_waits(bir_json):
    """This walrus build rejects instructions carrying attached semaphore waits
    ("Too many sync wait commands"); raw-bass style standalone EventSemaphore
    waits compile and run. Hoist every attached wait into its own
    EventSemaphore on the same engine queue; updates stay attached."""
    import json
    d = json.loads(bir_json)
    n = 0
    for fn in d["functions"]:
        for bb in fn["blocks"]:
            out = []
            for ins in bb["instructions"]:
                si = ins.get("sync_info")
                waits = (si or {}).get("on_wait") or []
                if waits and ins.get("opcode") != "EventSemaphore" and ins.get("engine"):
                    for w in waits:
                        n += 1
                        out.append({
                            "name": f"hoistw-{n}", "opcode": "EventSemaphore",
                            "engine": ins["engine"], "ins": [], "outs": [],
                            "sync_info": {"on_wait": [w], "on_update": []},
                        })
                    si["on_wait"] = []
                out.append(ins)
            bb["instructions"] = out
    return json.dumps(d).encode()


def _install_hoist(nc):
    orig = nc.to_json_bytes
    nc.to_json_bytes = lambda: _hoist_waits(orig())
    return nc


def _tables():
    t = np.arange(T)
    # BDCS [128,128]: rows (w4,t), cols (jj,w4); jj: 0=re u0, 1=re u16,
    # 2..16=re u=1..15, 17..31=im u=1..15
    blk = np.zeros((32, 32))
    blk[:, 0] = 1.0
    blk[:, 1] = np.cos(np.pi * t)
    for u in range(1, 16):
        blk[:, u + 1] = np.cos(2 * np.pi * t * u / 32.0)
        blk[:, u + 16] = -np.sin(2 * np.pi * t * u / 32.0)
    bdcs = np.zeros((128, 128))
    for w in range(4):
        bdcs[w * 32:(w + 1) * 32, w * 32:(w + 1) * 32] = blk
    perm = np.array([w4 * 32 + jj for jj in range(32) for w4 in range(4)])
    bdcs = bdcs[:, perm]

    s = np.arange(S)[:, None]
    v = np.arange(NV)[None, :]
    # ss [128, 62*128]: q=1..15 -> blocks (q-1)*4 + {SR1,SR2,SI1,SI2}; S0=60, S16=61
    ss = np.zeros((128, 62 * 128))
    for q in range(1, 16):
        phA = 2 * np.pi * s * (q + 32 * v) / 4096.0
        phB = 2 * np.pi * s * ((32 - q) + 32 * v) / 4096.0
        o = (q - 1) * 4 * 128
        ss[:, o:o + 128] = np.hstack([np.cos(phA), np.cos(phB)])         # SR1 @ rre
        ss[:, o + 128:o + 256] = np.hstack([np.sin(phA), -np.sin(phB)])   # SR2 @ rim
        ss[:, o + 256:o + 384] = np.hstack([-np.sin(phA), -np.sin(phB)])  # SI1 @ rre
        ss[:, o + 384:o + 512] = np.hstack([np.cos(phA), -np.cos(phB)])   # SI2 @ rim
    ph0 = 2 * np.pi * s * (32 * (v + 1)) / 4096.0
    ph16 = 2 * np.pi * s * (16 + 32 * v) / 4096.0
    ss[:, 60 * 128:61 * 128] = np.hstack([np.cos(ph0), -np.sin(ph0)])
    ss[:, 61 * 128:62 * 128] = np.hstack([np.cos(ph16), -np.sin(ph16)])

    vv = np.arange(NV)

    def ppcol(k):  # [64, 8]
        wk = np.where(k == 2048, 1.0, 2.0)
        return (wk[:, None] * 0.5 *
                np.cos(2 * np.pi * k[:, None] * DELAYS[None, :] / 4096.0) / 4096.0)

    ppj = np.zeros((128, 15 * 8))
    for q in range(1, 16):
        ppj[0:64, (q - 1) * 8:q * 8] = ppcol(q + 32 * vv)
        ppj[64:128, (q - 1) * 8:q * 8] = ppcol((32 - q) + 32 * vv)
    pp016 = np.zeros((64, 16))
    pp016[:, 0:8] = ppcol(32 * (vv + 1))
    pp016[:, 8:16] = ppcol(16 + 32 * vv)

    i64b = np.zeros((128, 64))
    i64b[np.arange(128), np.arange(128) % 64] = 1.0
    ident8 = np.eye(8)
    bf = ml_dtypes.bfloat16
    return (bdcs.astype(bf), ss.astype(bf), ppj.astype(bf), pp016.astype(bf),
            i64b.astype(bf), ident8.astype(np.float32))


def _build():
    nc = bass.Bass()
    audio = nc.dram_tensor("audio", [ITERS * 128, G * 128], BF16, kind="ExternalInput")
    bdcs_d = nc.dram_tensor("bdcs", [128, 128], BF16, kind="ExternalInput")
    ss_d = nc.dram_tensor("ss", [128, 62 * 128], BF16, kind="ExternalInput")
    ppj_d = nc.dram_tensor("ppj", [128, 120], BF16, kind="ExternalInput")
    pp016_d = nc.dram_tensor("pp016", [64, 16], BF16, kind="ExternalInput")
    oh_d = nc.dram_tensor("onehot", [128, 64], FP32, kind="ExternalInput")
    i64_d = nc.dram_tensor("i64b", [128, 64], BF16, kind="ExternalInput")
    id8_d = nc.dram_tensor("ident8", [8, 8], FP32, kind="ExternalInput")
    loss_out = nc.dram_tensor("loss_out", [128, 8], FP32, kind="ExternalOutput")
    cep_dbg = nc.dram_tensor("cep_dbg", [128, 64], FP32, kind="ExternalOutput")

    with tile.TileContext(nc) as tc:
        with (
            tc.tile_pool(name="consts", bufs=1) as consts,
            tc.tile_pool(name="xt", bufs=2) as xt_pool,
            tc.tile_pool(name="at", bufs=2) as at_pool,
            tc.tile_pool(name="sq", bufs=3) as sq_pool,
            tc.tile_pool(name="m2", bufs=3) as m2_pool,
            tc.tile_pool(name="lg", bufs=6) as lg_pool,
            tc.tile_pool(name="lg0", bufs=2) as lg0_pool,
            tc.tile_pool(name="fin", bufs=2) as fin_pool,
            tc.tile_pool(name="psA", bufs=2, space="PSUM") as psA_pool,
            tc.tile_pool(name="psX", bufs=2, space="PSUM") as psX_pool,
            tc.tile_pool(name="cep", bufs=1, space="PSUM") as cep_pool,
        ):
            bdcs = consts.tile([128, 128], BF16, tag="bdcs")
            nc.sync.dma_start(bdcs[:], bdcs_d[:])
            ss = consts.tile([128, 62 * 128], BF16, tag="ss")
            nc.sync.dma_start(ss[:], ss_d[:])
            ppj = consts.tile([128, 120], BF16, tag="ppj")
            nc.sync.dma_start(ppj[:], ppj_d[:])
            pp016 = consts.tile([64, 16], BF16, tag="pp016")
            nc.sync.dma_start(pp016[:], pp016_d[:])
            onehot = consts.tile([128, 64], FP32, tag="onehot")
            nc.sync.dma_start(onehot[:], oh_d[:])
            i64b = consts.tile([128, 64], BF16, tag="i64b")
            nc.sync.dma_start(i64b[:], i64_d[:])
            ident8 = consts.tile([8, 8], FP32, tag="ident8")
            nc.sync.dma_start(ident8[:], id8_d[:])
            epsb = consts.tile([128, 1], FP32, tag="epsb")
            nc.vector.memset(epsb[:], LN_EPS)
            cepT = consts.tile([128, 64], FP32, tag="cepT")

            def sblk(b):  # stationary block b of ss
                return ss[:, b * 128:(b + 1) * 128]

            xts = [None, None]
            ats = [None, None]

            def dma_xt(it):
                xt = xt_pool.tile([128, G * 128], BF16, tag="xt")
                for c in range(4):
                    nc.sync.dma_start(
                        xt[:, c * 2048:(c + 1) * 2048],
                        audio[it * 128:(it + 1) * 128, c * 2048:(c + 1) * 2048])
                xts[it % 2] = xt
                ats[it % 2] = at_pool.tile([128, G * 128], BF16, tag="at", name="at")

            def quad(it, qd):
                # stage1: 4 groups -> psA [s, (g, j, w4)], scatter-copy j-major
                xt, at = xts[it % 2], ats[it % 2]
                g0 = qd * 4
                psA = psA_pool.tile([128, 512], FP32, tag="psA")
                for g in range(g0, g0 + 4):
                    nc.tensor.matmul(psA[:, (g - g0) * 128:(g - g0 + 1) * 128],
                                     xt[:, g * 128:(g + 1) * 128], bdcs[:],
                                     start=True, stop=True)
                atv_j = at[:].rearrange("s (j g w4) -> s j g w4", j=32, w4=4)
                dst = atv_j[:, :, g0:g0 + 4, :]
                src = psA[:].rearrange("s (g j w4) -> s j g w4", g=4, w4=4)
                nc.vector.tensor_copy(dst, src)

            class IterCtx:
                pass

            def start_iter(it):
                ctx = IterCtx()
                ctx.it = it
                ctx.at = ats[it % 2]
                ctx.cep = cep_pool.tile([128, 512], FP32, tag="cep", name="cep")
                ctx.projq = []
                ctx.emitted = 0
                ctx.sq7 = None
                return ctx

            def cep_acc(ctx, stat, lgt):
                # accumulating projection matmul; start on first, stop on 17th
                nc.tensor.matmul(ctx.cep[0:8, 0:256], stat, lgt,
                                 start=(ctx.emitted == 0),
                                 stop=(ctx.emitted == 16))
                ctx.emitted += 1

            def add_proj(ctx, stat, lgt):
                ctx.projq.append(lambda ctx=ctx, s=stat, l=lgt: cep_acc(ctx, s, l))

            def emit_proj(ctx):
                ctx.projq.pop(0)()

            def super_pair(ctx, s):
                # two pairs per psX bank-pair; s=7 packs (q=15, u0/u16)
                at = ctx.at
                psX = psX_pool.tile([128, 1024], FP32, tag="psX")
                if s < 7:
                    qa, qb = 2 * s + 1, 2 * s + 2
                    for h, q in ((0, qa), (1, qb)):
                        rre = at[:, (q + 1) * 256:(q + 2) * 256]
                        rim = at[:, (q + 16) * 256:(q + 17) * 256]
                        o = (q - 1) * 4
                        nc.tensor.matmul(psX[:, h * 512:h * 512 + 256],
                                         sblk(o), rre, start=True, stop=False)
                        nc.tensor.matmul(psX[:, h * 512:h * 512 + 256],
                                         sblk(o + 1), rim, start=False, stop=True)
                        nc.tensor.matmul(psX[:, h * 512 + 256:h * 512 + 512],
                                         sblk(o + 2), rre, start=True, stop=False)
                        nc.tensor.matmul(psX[:, h * 512 + 256:h * 512 + 512],
                                         sblk(o + 3), rim, start=False, stop=True)
                else:
                    q = 15
                    rre = at[:, (q + 1) * 256:(q + 2) * 256]
                    rim = at[:, (q + 16) * 256:(q + 17) * 256]
                    o = (q - 1) * 4
                    nc.tensor.matmul(psX[:, 0:256], sblk(o), rre, start=True, stop=False)
                    nc.tensor.matmul(psX[:, 0:256], sblk(o + 1), rim, start=False, stop=True)
                    nc.tensor.matmul(psX[:, 256:512], sblk(o + 2), rre, start=True, stop=False)
                    nc.tensor.matmul(psX[:, 256:512], sblk(o + 3), rim, start=False, stop=True)
                    nc.tensor.matmul(psX[:, 512:768], sblk(60), at[:, 0:256],
                                     start=True, stop=True)
                    nc.tensor.matmul(psX[:, 768:1024], sblk(61), at[:, 256:512],
                                     start=True, stop=True)
                sq = sq_pool.tile([128, 1024], BF16, tag="sq")
                nc.scalar.activation(sq[:], psX[:],
                                     mybir.ActivationFunctionType.Square,
                                     scale=SQ_SCALE)
                m2 = m2_pool.tile([128, 512], BF16, tag="m2")
                if s < 7:
                    nc.gpsimd.tensor_add(m2[:, 0:256], sq[:, 0:256], sq[:, 256:512])
                    nc.gpsimd.tensor_add(m2[:, 256:512], sq[:, 512:768], sq[:, 768:1024])
                    lg = lg_pool.tile([128, 512], BF16, tag="lg")
                    nc.scalar.activation(lg[:], m2[:],
                                         mybir.ActivationFunctionType.Ln,
                                         bias=epsb[:])
                    add_proj(ctx, ppj[:, (qa - 1) * 8:qa * 8], lg[:, 0:256])
                    add_proj(ctx, ppj[:, (qb - 1) * 8:qb * 8], lg[:, 256:512])
                else:
                    nc.gpsimd.tensor_add(m2[:, 0:256], sq[:, 0:256], sq[:, 256:512])
                    lg = lg_pool.tile([128, 512], BF16, tag="lg")
                    nc.scalar.activation(lg[:, 0:256], m2[:, 0:256],
                                         mybir.ActivationFunctionType.Ln,
                                         bias=epsb[:])
                    add_proj(ctx, ppj[:, 14 * 8:15 * 8], lg[:, 0:256])
                    ctx.sq7 = sq

            def q0_tail(ctx):
                sq = ctx.sq7
                psM = psX_pool.tile([128, 1024], FP32, tag="psX")
                nc.tensor.matmul(psM[0:64, 0:512], i64b[:], sq[:, 512:1024],
                                 start=True, stop=True)
                lg0 = lg0_pool.tile([64, 512], BF16, tag="lg0")
                nc.scalar.activation(lg0[:], psM[0:64, 0:512],
                                     mybir.ActivationFunctionType.Ln,
                                     bias=epsb[0:64])
                cep_acc(ctx, pp016[:, 0:8], lg0[:, 0:256])
                cep_acc(ctx, pp016[:, 8:16], lg0[:, 256:512])

            def fin_tail(ctx):
                cep_sb = fin_pool.tile([8, 256], FP32, tag="cep_sb")
                nc.scalar.activation(cep_sb[:], ctx.cep[0:8, 0:256],
                                     mybir.ActivationFunctionType.Copy)
                for c in range(2):
                    gc = ctx.it * 2 + c
                    psC = ctx.cep[:, 256 + c * 8:256 + (c + 1) * 8]
                    nc.tensor.transpose(psC, cep_sb[:, c * 128:(c + 1) * 128],
                                        ident8[:])
                    nc.vector.tensor_copy(cepT[:, gc * 8:(gc + 1) * 8], psC)

            # ---- prologue: iteration 0 stage1 ----
            dma_xt(0)
            for qd in range(16):
                quad(0, qd)

            prev = None
            for it in range(ITERS):
                ctx = start_iter(it)
                if it + 1 < ITERS:
                    dma_xt(it + 1)
                for s in range(8):
                    super_pair(ctx, s)
                    if it + 1 < ITERS:
                        quad(it + 1, 2 * s)
                        quad(it + 1, 2 * s + 1)
                    if prev is not None:
                        # previous iteration's tail, spread across early supers
                        if s == 0:
                            emit_proj(prev)
                            emit_proj(prev)
                            emit_proj(prev)
                        elif s == 1:
                            q0_tail(prev)
                        elif s == 2:
                            fin_tail(prev)
                    if s >= 2:
                        emit_proj(ctx)
                        emit_proj(ctx)
                prev = ctx
            while prev.projq:
                emit_proj(prev)
            q0_tail(prev)
            fin_tail(prev)

            # batched loss over all 1024 windows: [128 w, 8 audios]
            tmp = fin_pool.tile([128, 64], FP32, tag="tmp")
            nc.vector.tensor_mul(tmp[:], cepT[:], onehot[:])
            sel = fin_pool.tile([128, 8], FP32, tag="sel")
            nc.vector.reduce_sum(sel[:], tmp[:].rearrange("p (a j) -> p a j", j=8),
                                 axis=mybir.AxisListType.X)
            mx = fin_pool.tile([128, 8], FP32, tag="mx")
            nc.vector.reduce_max(mx[:], cepT[:].rearrange("p (a j) -> p a j", j=8),
                                 axis=mybir.AxisListType.X)
            df = fin_pool.tile([128, 8], FP32, tag="df")
            nc.vector.tensor_sub(df[:], mx[:], sel[:])
            df2 = fin_pool.tile([128, 8], FP32, tag="df2")
            nc.vector.tensor_scalar_mul(df2[:], df[:], 1e12)
            ls = fin_pool.tile([128, 8], FP32, tag="ls")
            nc.vector.tensor_scalar_min(ls[:], df2[:], 1.0)
            nc.sync.dma_start(loss_out[:], ls[:])
            nc.sync.dma_start(cep_dbg[:], cepT[:])
    return nc


def kernel(audio_batch, symbols_batch, num_errs_no_reverb_batch,
           num_errs_reverb_batch):
    audio_batch = np.asarray(audio_batch)
    symbols_batch = np.asarray(symbols_batch, dtype=np.int32)
    nn_ = np.asarray(num_errs_no_reverb_batch).astype(np.float32)
    nr_ = np.asarray(num_errs_reverb_batch).astype(np.float32)

    if "nc" not in _cache:
        _cache["nc"] = _install_hoist(_build())
        _cache["tabs"] = _tables()
    nc = _cache["nc"]
    bdcs, ss, ppj, pp016, i64b, ident8 = _cache["tabs"]

    # host pre-transpose: [core][it, (w4 t), (g s)] so device DMA is contiguous
    wins = (audio_batch.reshape(NCORES, WLOC, T, S)
            .reshape(NCORES, ITERS, G, 4, T, S)
            .transpose(0, 1, 3, 4, 2, 5)
            .reshape(NCORES, ITERS * 128, G * 128)
            .astype(ml_dtypes.bfloat16))
    sy = symbols_batch.reshape(NCORES, BLOC, NW)
    in_maps = []
    for c in range(NCORES):
        oh = (sy[c].T[:, :, None] == np.arange(8)).astype(np.float32).reshape(128, 64)
        in_maps.append({
            "audio": wins[c], "onehot": oh,
            "bdcs": bdcs, "ss": ss, "ppj": ppj, "pp016": pp016,
            "i64b": i64b, "ident8": ident8,
        })
    import os
    res = run_bass_kernel_spmd(nc, in_maps, core_ids=list(range(NCORES)),
                               trace=bool(os.environ.get("KTRACE")))
    _cache["last_res"] = res
    errs = np.zeros(B, np.float32)
    for c in range(NCORES):
        loss = res.results[c]["loss_out"]          # [128 w, 8 audios]
        errs[c * BLOC:(c + 1) * BLOC] = loss.sum(axis=0, dtype=np.float32)

    tot = np.float32(errs.sum())
    diff = nr_ - nn_
    inv_red = np.where(diff == 0, np.float32(1.0), diff / (nr_ - errs))
    ter = np.float32(inv_red.sum())
    denom = np.float32(B * NW)
    return (np.float32(tot / denom), tot, np.float32(ter / B),
            np.float32(nn_.sum() / denom), np.float32(nr_.sum() / denom))


# revision 35
# speedup vs baseline: 4.2283x; 1.0514x over previous
"""Trainium2 Bass kernel for nn_DecodingLoss (cepstrum decoding loss).

Math (per 4096-sample window):
  cep = irfft(log(|rfft(x)| + eps))[DELAYS]; softargmax(beta=1e10) ~= hard argmax;
  loss = clip(|idx - symbol|,0,1) = 1[argmax != symbol]; per-audio sums -> 5 scalars.

Kernel strategy (8 cores, pure data parallel over the batch dim; 1024 windows/core):
  FFT 4096 = 32 x 128 Cooley-Tukey, n = 128 t + s (t<32, s<128), k = u + 32 v.
  stage1 (PE): per 4-window group, stationary = x4 [(w4 t), s], moving = block-diag
    W32 table -> psA = A^T[s, (j, w4)] directly (no transpose step). Real-input
    hermitian symmetry: only u=0..16 kept (u0/u16 real), 32 j-cols per window.
    The PSUM->SBUF copy scatters to a j-major `at` so every stage2 moving operand
    is a contiguous 256-column slice (strided movings halve PE stream rate).
  stage2 (PE): for q=1..15 the conjugate k-sets {q+32v} and {32-q+32v} share
    moving operands rre/rim; 128-wide stationaries emit Re of both sets into one
    psX region and Im into another -> |X|^2 = aligned full-width adds. Two q's
    batched per [128,1024] psX ("super-pair") to halve ACT instruction count.
    u=0/16 singleton handled via a PE stacked-identity sum (psM).
  log|X|: ACT Square(scale 2^-6) -> bf16, DVE add, ACT Ln (values centered near 0
  so bf16 is safe), per-pair bf16 projection matmuls accumulate cep[8, 256 win].
  Loss: transpose cep to [win, tap]; batched: sel = cep[sym] via one-hot mult,
  loss = min((max - sel)*1e12, 1). Host sums per-audio errors + final scalars.
  Pipelining: stage1 quads of iteration N+1 are interleaved between the supers of
  iteration N so copies never gate the PE; projections trail their super by 2.
"""
import numpy as np
import ml_dtypes

import concourse.bass as bass
import concourse.mybir as mybir
from concourse import tile
from concourse.bass_utils import run_bass_kernel_spmd

FP32 = mybir.dt.float32
BF16 = mybir.dt.bfloat16
F8 = mybir.dt.float8e4
I32 = mybir.dt.int32
F8NP = ml_dtypes.float8_e4m3fn

B, NW, WIN = 64, 128, 4096
NCORES = 8
BLOC = B // NCORES              # 8 audio rows per core
WLOC = BLOC * NW                # 1024 windows per core
T, S = 32, 128                  # n = 128 t + s
NV = 64                         # v-grid size per k-set
ITERS = 4
WPI = WLOC // ITERS             # 256 windows per iteration
G = WPI // 4                    # 64 groups of 4 windows
DELAYS = np.array([64, 96, 128, 160, 192, 224, 256, 288])
SQ_SCALE = 2.0 ** -6            # |X|^2 scaled by 2^-12: ln output centered near 0
LN_EPS = 2.44e-14

_cache = {}


def _hoist_waits(bir_json):
    """This walrus build rejects instructions carrying attached semaphore waits
    ("Too many sync wait commands"); raw-bass style standalone EventSemaphore
    waits compile and run. Hoist every attached wait into its own
    EventSemaphore on the same engine queue; updates stay attached."""
    import json
    d = json.loads(bir_json)
    n = 0
    for fn in d["functions"]:
        for bb in fn["blocks"]:
            out = []
            for ins in bb["instructions"]:
                si = ins.get("sync_info")
                waits = (si or {}).get("on_wait") or []
                if waits and ins.get("opcode") != "EventSemaphore" and ins.get("engine"):
                    for w in waits:
                        n += 1
                        out.append({
                            "name": f"hoistw-{n}", "opcode": "EventSemaphore",
                            "engine": ins["engine"], "ins": [], "outs": [],
                            "sync_info": {"on_wait": [w], "on_update": []},
                        })
                    si["on_wait"] = []
                out.append(ins)
            bb["instructions"] = out
    return json.dumps(d).encode()


def _install_hoist(nc):
    orig = nc.to_json_bytes
    nc.to_json_bytes = lambda: _hoist_waits(orig())
    return nc


def _tables():
    t = np.arange(T)
    # BDCS [128,128]: rows (w4,t), cols (jj,w4); jj: 0=re u0, 1=re u16,
    # 2..16=re u=1..15, 17..31=im u=1..15
    blk = np.zeros((32, 32))
    blk[:, 0] = 1.0
    blk[:, 1] = np.cos(np.pi * t)
    for u in range(1, 16):
        blk[:, u + 1] = np.cos(2 * np.pi * t * u / 32.0)
        blk[:, u + 16] = -np.sin(2 * np.pi * t * u / 32.0)
    bdcs = np.zeros((128, 128))
    for w in range(4):
        bdcs[w * 32:(w + 1) * 32, w * 32:(w + 1) * 32] = blk
    # j' layout: 0=u0re, 1=u16re, 2q=re(q), 2q+1=im(q) -- re/im adjacent so the
    # stage2 DoubleRow moving operand is one [s, 2, 256] view
    jj_old = [0, 1] + [q + 1 if c == 0 else q + 16
                       for q in range(1, 16) for c in range(2)]
    perm = np.array([w4 * 32 + jj_old[jp] for jp in range(32) for w4 in range(4)])
    bdcs = bdcs[:, perm]

    s = np.arange(S)[:, None]
    v = np.arange(NV)[None, :]
    # ss [128, 62*128]: q=1..15 -> blocks (q-1)*4 + {SR1,SR2,SI1,SI2}; S0=60, S16=61
    ss = np.zeros((128, 62 * 128))
    for q in range(1, 16):
        phA = 2 * np.pi * s * (q + 32 * v) / 4096.0
        phB = 2 * np.pi * s * ((32 - q) + 32 * v) / 4096.0
        o = (q - 1) * 4 * 128
        ss[:, o:o + 128] = np.hstack([np.cos(phA), np.cos(phB)])         # SR1 @ rre
        ss[:, o + 128:o + 256] = np.hstack([np.sin(phA), -np.sin(phB)])   # SR2 @ rim
        ss[:, o + 256:o + 384] = np.hstack([-np.sin(phA), -np.sin(phB)])  # SI1 @ rre
        ss[:, o + 384:o + 512] = np.hstack([np.cos(phA), -np.cos(phB)])   # SI2 @ rim
    ph0 = 2 * np.pi * s * (32 * (v + 1)) / 4096.0
    ph16 = 2 * np.pi * s * (16 + 32 * v) / 4096.0
    ss[:, 60 * 128:61 * 128] = np.hstack([np.cos(ph0), -np.sin(ph0)])
    ss[:, 61 * 128:62 * 128] = np.hstack([np.cos(ph16), -np.sin(ph16)])

    vv = np.arange(NV)

    def ppcol(k):  # [64, 8]
        wk = np.where(k == 2048, 1.0, 2.0)
        return (wk[:, None] * 0.5 *
                np.cos(2 * np.pi * k[:, None] * DELAYS[None, :] / 4096.0) / 4096.0)

    ppj = np.zeros((128, 15 * 8))
    for q in range(1, 16):
        ppj[0:64, (q - 1) * 8:q * 8] = ppcol(q + 32 * vv)
        ppj[64:128, (q - 1) * 8:q * 8] = ppcol((32 - q) + 32 * vv)
    pp016 = np.zeros((64, 16))
    pp016[:, 0:8] = ppcol(32 * (vv + 1))
    pp016[:, 8:16] = ppcol(16 + 32 * vv)

    i64b = np.zeros((128, 64))
    i64b[np.arange(128), np.arange(128) % 64] = 1.0
    ident8 = np.eye(8)
    bf = ml_dtypes.bfloat16
    return (bdcs.astype(bf), ss.astype(F8NP), ppj.astype(bf), pp016.astype(bf),
            i64b.astype(bf), ident8.astype(np.float32))


def _build():
    nc = bass.Bass()
    audio = nc.dram_tensor("audio", [ITERS * 128, G * 128], BF16, kind="ExternalInput")
    bdcs_d = nc.dram_tensor("bdcs", [128, 128], BF16, kind="ExternalInput")
    ss_d = nc.dram_tensor("ss", [128, 62 * 128], F8, kind="ExternalInput")
    ppj_d = nc.dram_tensor("ppj", [128, 120], BF16, kind="ExternalInput")
    pp016_d = nc.dram_tensor("pp016", [64, 16], BF16, kind="ExternalInput")
    oh_d = nc.dram_tensor("onehot", [128, 64], FP32, kind="ExternalInput")
    i64_d = nc.dram_tensor("i64b", [128, 64], BF16, kind="ExternalInput")
    id8_d = nc.dram_tensor("ident8", [8, 8], FP32, kind="ExternalInput")
    loss_out = nc.dram_tensor("loss_out", [128, 8], FP32, kind="ExternalOutput")
    cep_dbg = nc.dram_tensor("cep_dbg", [128, 64], FP32, kind="ExternalOutput")

    with tile.TileContext(nc) as tc:
        with (
            tc.tile_pool(name="consts", bufs=1) as consts,
            tc.tile_pool(name="xt", bufs=2) as xt_pool,
            tc.tile_pool(name="at", bufs=2) as at_pool,
            tc.tile_pool(name="sq", bufs=3) as sq_pool,
            tc.tile_pool(name="m2", bufs=3) as m2_pool,
            tc.tile_pool(name="lg", bufs=6) as lg_pool,
            tc.tile_pool(name="lg0", bufs=2) as lg0_pool,
            tc.tile_pool(name="fin", bufs=2) as fin_pool,
            tc.tile_pool(name="psA", bufs=2, space="PSUM") as psA_pool,
            tc.tile_pool(name="psX", bufs=2, space="PSUM") as psX_pool,
            tc.tile_pool(name="cep", bufs=1, space="PSUM") as cep_pool,
        ):
            bdcs = consts.tile([128, 128], BF16, tag="bdcs")
            nc.sync.dma_start(bdcs[:], bdcs_d[:])
            ss = consts.tile([128, 62 * 128], F8, tag="ss")
            nc.sync.dma_start(ss[:], ss_d[:])
            ppj = consts.tile([128, 120], BF16, tag="ppj")
            nc.sync.dma_start(ppj[:], ppj_d[:])
            pp016 = consts.tile([64, 16], BF16, tag="pp016")
            nc.sync.dma_start(pp016[:], pp016_d[:])
            onehot = consts.tile([128, 64], FP32, tag="onehot")
            nc.sync.dma_start(onehot[:], oh_d[:])
            i64b = consts.tile([128, 64], BF16, tag="i64b")
            nc.sync.dma_start(i64b[:], i64_d[:])
            ident8 = consts.tile([8, 8], FP32, tag="ident8")
            nc.sync.dma_start(ident8[:], id8_d[:])
            epsb = consts.tile([128, 1], FP32, tag="epsb")
            nc.vector.memset(epsb[:], LN_EPS)
            cepT = consts.tile([128, 64], FP32, tag="cepT")

            def sblk(b):  # stationary block b of ss
                return ss[:, b * 128:(b + 1) * 128]

            xts = [None, None]
            ats = [None, None]

            def dma_xt(it):
                xt = xt_pool.tile([128, G * 128], BF16, tag="xt")
                for c in range(4):
                    nc.sync.dma_start(
                        xt[:, c * 2048:(c + 1) * 2048],
                        audio[it * 128:(it + 1) * 128, c * 2048:(c + 1) * 2048])
                xts[it % 2] = xt
                ats[it % 2] = at_pool.tile([128, G * 128], F8, tag="at", name="at")

            def quad(it, qd):
                # stage1: 4 groups -> psA [s, (g, j, w4)], scatter-copy j-major
                xt, at = xts[it % 2], ats[it % 2]
                g0 = qd * 4
                psA = psA_pool.tile([128, 512], FP32, tag="psA")
                for g in range(g0, g0 + 4):
                    nc.tensor.matmul(psA[:, (g - g0) * 128:(g - g0 + 1) * 128],
                                     xt[:, g * 128:(g + 1) * 128], bdcs[:],
                                     start=True, stop=True)
                atv_j = at[:].rearrange("s (j g w4) -> s j g w4", j=32, w4=4)
                dst = atv_j[:, :, g0:g0 + 4, :]
                src = psA[:].rearrange("s (g j w4) -> s j g w4", g=4, w4=4)
                nc.vector.tensor_copy(dst, src)

            class IterCtx:
                pass

            def start_iter(it):
                ctx = IterCtx()
                ctx.it = it
                ctx.at = ats[it % 2]
                ctx.cep = cep_pool.tile([128, 512], FP32, tag="cep", name="cep")
                ctx.projq = []
                ctx.emitted = 0
                ctx.sq7 = None
                return ctx

            def cep_acc(ctx, stat, lgt):
                # accumulating projection matmul; start on first, stop on 17th
                nc.tensor.matmul(ctx.cep[0:8, 0:256], stat, lgt,
                                 start=(ctx.emitted == 0),
                                 stop=(ctx.emitted == 16))
                ctx.emitted += 1

            def add_proj(ctx, stat, lgt):
                ctx.projq.append(lambda ctx=ctx, s=stat, l=lgt: cep_acc(ctx, s, l))

            def emit_proj(ctx):
                ctx.projq.pop(0)()

            def super_pair(ctx, s):
                # two pairs per psX bank-pair; s=7 packs (q=15, u0/u16)
                at = ctx.at
                psX = psX_pool.tile([128, 1024], FP32, tag="psX")
                DR = mybir.MatmulPerfMode.DoubleRow

                def pair(q, h):
                    # one DoubleRow matmul per region: 2x128-deep contraction
                    # over (s, re/im) with stationary [SR1|SR2] / [SI1|SI2]
                    rr2 = (at[:, 2 * q * 256:(2 * q + 2) * 256]
                           .rearrange("s (two w) -> s two w", two=2))
                    o = (q - 1) * 4
                    wre = (ss[:, o * 128:(o + 2) * 128]
                           .rearrange("s (two f) -> s two f", two=2))
                    wim = (ss[:, (o + 2) * 128:(o + 4) * 128]
                           .rearrange("s (two f) -> s two f", two=2))
                    nc.tensor.matmul(psX[:, h * 512:h * 512 + 256], wre, rr2,
                                     start=True, stop=True, perf_mode=DR)
                    nc.tensor.matmul(psX[:, h * 512 + 256:h * 512 + 512], wim, rr2,
                                     start=True, stop=True, perf_mode=DR)

                if s < 7:
                    qa, qb = 2 * s + 1, 2 * s + 2
                    pair(qa, 0)
                    pair(qb, 1)
                else:
                    pair(15, 0)
                    nc.tensor.matmul(psX[:, 512:768], sblk(60), at[:, 0:256],
                                     start=True, stop=True)
                    nc.tensor.matmul(psX[:, 768:1024], sblk(61), at[:, 256:512],
                                     start=True, stop=True)
                sq = sq_pool.tile([128, 1024], BF16, tag="sq")
                nc.scalar.activation(sq[:], psX[:],
                                     mybir.ActivationFunctionType.Square,
                                     scale=SQ_SCALE)
                m2 = m2_pool.tile([128, 512], BF16, tag="m2")
                if s < 7:
                    nc.gpsimd.tensor_add(m2[:, 0:256], sq[:, 0:256], sq[:, 256:512])
                    nc.gpsimd.tensor_add(m2[:, 256:512], sq[:, 512:768], sq[:, 768:1024])
                    lg = lg_pool.tile([128, 512], BF16, tag="lg")
                    nc.scalar.activation(lg[:], m2[:],
                                         mybir.ActivationFunctionType.Ln,
                                         bias=epsb[:])
                    add_proj(ctx, ppj[:, (qa - 1) * 8:qa * 8], lg[:, 0:256])
                    add_proj(ctx, ppj[:, (qb - 1) * 8:qb * 8], lg[:, 256:512])
                else:
                    nc.gpsimd.tensor_add(m2[:, 0:256], sq[:, 0:256], sq[:, 256:512])
                    lg = lg_pool.tile([128, 512], BF16, tag="lg")
                    nc.scalar.activation(lg[:, 0:256], m2[:, 0:256],
                                         mybir.ActivationFunctionType.Ln,
                                         bias=epsb[:])
                    add_proj(ctx, ppj[:, 14 * 8:15 * 8], lg[:, 0:256])
                    ctx.sq7 = sq

            def q0_tail(ctx):
                sq = ctx.sq7
                psM = psX_pool.tile([128, 1024], FP32, tag="psX")
                nc.tensor.matmul(psM[0:64, 0:512], i64b[:], sq[:, 512:1024],
                                 start=True, stop=True)
                lg0 = lg0_pool.tile([64, 512], BF16, tag="lg0")
                nc.scalar.activation(lg0[:], psM[0:64, 0:512],
                                     mybir.ActivationFunctionType.Ln,
                                     bias=epsb[0:64])
                cep_acc(ctx, pp016[:, 0:8], lg0[:, 0:256])
                cep_acc(ctx, pp016[:, 8:16], lg0[:, 256:512])

            def fin_tail(ctx):
                cep_sb = fin_pool.tile([8, 256], FP32, tag="cep_sb")
                nc.scalar.activation(cep_sb[:], ctx.cep[0:8, 0:256],
                                     mybir.ActivationFunctionType.Copy)
                for c in range(2):
                    gc = ctx.it * 2 + c
                    psC = ctx.cep[:, 256 + c * 8:256 + (c + 1) * 8]
                    nc.tensor.transpose(psC, cep_sb[:, c * 128:(c + 1) * 128],
                                        ident8[:])
                    nc.vector.tensor_copy(cepT[:, gc * 8:(gc + 1) * 8], psC)

            # ---- prologue: iteration 0 stage1 ----
            dma_xt(0)
            for qd in range(16):
                quad(0, qd)

            prev = None
            for it in range(ITERS):
                ctx = start_iter(it)
                if it + 1 < ITERS:
                    dma_xt(it + 1)
                for s in range(8):
                    super_pair(ctx, s)
                    if it + 1 < ITERS:
                        quad(it + 1, 2 * s)
                        quad(it + 1, 2 * s + 1)
                    if prev is not None:
                        # previous iteration's tail, spread across early supers
                        if s == 0:
                            emit_proj(prev)
                            emit_proj(prev)
                            emit_proj(prev)
                        elif s == 1:
                            q0_tail(prev)
                        elif s == 2:
                            fin_tail(prev)
                    if s >= 2:
                        emit_proj(ctx)
                        emit_proj(ctx)
                prev = ctx
            while prev.projq:
                emit_proj(prev)
            q0_tail(prev)
            fin_tail(prev)

            # batched loss over all 1024 windows: [128 w, 8 audios]
            tmp = fin_pool.tile([128, 64], FP32, tag="tmp")
            nc.vector.tensor_mul(tmp[:], cepT[:], onehot[:])
            sel = fin_pool.tile([128, 8], FP32, tag="sel")
            nc.vector.reduce_sum(sel[:], tmp[:].rearrange("p (a j) -> p a j", j=8),
                                 axis=mybir.AxisListType.X)
            mx = fin_pool.tile([128, 8], FP32, tag="mx")
            nc.vector.reduce_max(mx[:], cepT[:].rearrange("p (a j) -> p a j", j=8),
                                 axis=mybir.AxisListType.X)
            df = fin_pool.tile([128, 8], FP32, tag="df")
            nc.vector.tensor_sub(df[:], mx[:], sel[:])
            df2 = fin_pool.tile([128, 8], FP32, tag="df2")
            nc.vector.tensor_scalar_mul(df2[:], df[:], 1e12)
            ls = fin_pool.tile([128, 8], FP32, tag="ls")
            nc.vector.tensor_scalar_min(ls[:], df2[:], 1.0)
            nc.sync.dma_start(loss_out[:], ls[:])
            nc.sync.dma_start(cep_dbg[:], cepT[:])
    return nc


def kernel(audio_batch, symbols_batch, num_errs_no_reverb_batch,
           num_errs_reverb_batch):
    audio_batch = np.asarray(audio_batch)
    symbols_batch = np.asarray(symbols_batch, dtype=np.int32)
    nn_ = np.asarray(num_errs_no_reverb_batch).astype(np.float32)
    nr_ = np.asarray(num_errs_reverb_batch).astype(np.float32)

    if "nc" not in _cache:
        _cache["nc"] = _install_hoist(_build())
        _cache["tabs"] = _tables()
    nc = _cache["nc"]
    bdcs, ss, ppj, pp016, i64b, ident8 = _cache["tabs"]

    # host pre-transpose: [core][it, (w4 t), (g s)] so device DMA is contiguous
    wins = (audio_batch.reshape(NCORES, WLOC, T, S)
            .reshape(NCORES, ITERS, G, 4, T, S)
            .transpose(0, 1, 3, 4, 2, 5)
            .reshape(NCORES, ITERS * 128, G * 128)
            .astype(ml_dtypes.bfloat16))
    sy = symbols_batch.reshape(NCORES, BLOC, NW)
    in_maps = []
    for c in range(NCORES):
        oh = (sy[c].T[:, :, None] == np.arange(8)).astype(np.float32).reshape(128, 64)
        in_maps.append({
            "audio": wins[c], "onehot": oh,
            "bdcs": bdcs, "ss": ss, "ppj": ppj, "pp016": pp016,
            "i64b": i64b, "ident8": ident8,
        })
    import os
    res = run_bass_kernel_spmd(nc, in_maps, core_ids=list(range(NCORES)),
                               trace=bool(os.environ.get("KTRACE")))
    _cache["last_res"] = res
    errs = np.zeros(B, np.float32)
    for c in range(NCORES):
        loss = res.results[c]["loss_out"]          # [128 w, 8 audios]
        errs[c * BLOC:(c + 1) * BLOC] = loss.sum(axis=0, dtype=np.float32)

    tot = np.float32(errs.sum())
    diff = nr_ - nn_
    inv_red = np.where(diff == 0, np.float32(1.0), diff / (nr_ - errs))
    ter = np.float32(inv_red.sum())
    denom = np.float32(B * NW)
    return (np.float32(tot / denom), tot, np.float32(ter / B),
            np.float32(nn_.sum() / denom), np.float32(nr_.sum() / denom))
